# revision 36
# baseline (speedup 1.0000x reference)
"""Trainium2 Bass kernel for nn_EnhancedFlowLayer (topk_masking).

8 cores. Tokens on partitions (2 groups of 128); flow (i,j)-space sharded by i
across cores (64 i-rows -> 32768 elems/token/core). flow is rematerialized on
the PE per phase (quad-packed: 4 concurrent K=16 matmuls via tile_position)
and never hits HBM. Per-token exact rank-kk threshold via: bf16 |F| + sampled
Newton + exact 5-rung count ladder (counts split across DVE and ACT engines;
ACT counts use Sign at bf16-midpoint boundaries, tie-free and exact) + band
extraction (top-2 per 64-chunk, narrow band) + one all-gather + replicated
exact fp32 bisection (10 rounds, DVE/ACT split) + top-8 peel select. Final
pass recomputes F, applies mask, does the masked matvec, one all-gather of
flow_out slices, then a replicated LN2 + memory-MLP and an N-sharded FFN
(per-core output-column strip + AllReduce of down-proj partials).
"""

import os
from contextlib import ExitStack

import numpy as np

B, S, D, P = 1, 256, 512, 16
MAX_SEQ = 4096
NCORES = 8
ISLICE = D // NCORES          # 64 i-rows per core
FREE = ISLICE * D             # 32768 ij elements per token per core
NG = 2                        # token groups of 128
DD = D * D
TOPL = int(os.environ.get("KERNEL_TOPL", "2"))       # top-L per 64-chunk
BAND = float(os.environ.get("KERNEL_BAND", "450.0"))  # ladder band width (elems)
NCH = FREE // 64              # 512 chunks of 64
NCAND = TOPL * NCH            # candidate slots per token per core
NL = 5                        # ladder rungs
N_BISECT = int(os.environ.get("KERNEL_NBISECT", "8"))
# ladder/bisect engine split (elements handled by DVE vs ACT); all segments
# 8192-wide to stay 16KB-aligned (misaligned reads cost ~20% on DVE counts)
LAD_DVE = 16384               # of 32768 (2 DVE + 2 ACT segments of 8192)
LAD_ACT = (FREE - LAD_DVE) // 2
BIS_DVE = 3584                # bisect DVE count width (rest of 8192 on ACT)

DEBUG = os.environ.get("KERNEL_DEBUG", "0") == "1"
STAGE = int(os.environ.get("KERNEL_STAGE", "3"))
MM_DT_NAME = os.environ.get("KERNEL_MM_DT", "float32")
SIM_COMPAT = os.environ.get("KERNEL_SIM_COMPAT", "0") == "1"


def _host_constants():
    pos = np.arange(S, dtype=np.float64)
    inv = 1.0 / (10000.0 ** (np.arange(0, D, 2, dtype=np.float64) / D))
    ang = pos[:, None] * inv[None, :]
    sin = np.repeat(np.sin(ang), 2, axis=-1).astype(np.float32)
    cos = np.repeat(np.cos(ang), 2, axis=-1).astype(np.float32)
    # half-normal tail quantile z(q): P(|N(0,1)| >= z) = q, cubic in ln q
    qpoly = np.array([-0.0036756, -0.06789169, -0.73664117, 0.26370117], np.float32)
    return sin, cos, qpoly


def build_kernel():
    import concourse.bass as bass
    import concourse.mybir as mybir
    from concourse import bacc, masks
    from concourse.tile import TileContext

    dt = mybir.dt
    Alu = mybir.AluOpType
    Act = mybir.ActivationFunctionType
    AxX = mybir.AxisListType.X
    f32, bf16 = dt.float32, dt.bfloat16
    MM_DT = getattr(dt, MM_DT_NAME)

    nc = bacc.Bacc("TRN2", num_devices=NCORES)

    def mmc(ap):
        return ap.bitcast(MM_DT) if MM_DT != f32 else ap

    dp = nc.declare_dram_parameter
    x_in = dp("x", [S, D], f32, isOutput=False)
    pat_sl = dp("pat_sl", [P, FREE], f32, isOutput=False)
    sel_w1 = dp("sel_w1", [2 * D, 2 * P], f32, isOutput=False)
    sel_b1 = dp("sel_b1", [1, 2 * P], f32, isOutput=False)
    sel_w2 = dp("sel_w2", [2 * P, P], f32, isOutput=False)
    sel_b2 = dp("sel_b2", [1, P], f32, isOutput=False)
    win_w1 = dp("win_w1", [D, 64], f32, isOutput=False)
    win_b1 = dp("win_b1", [1, 64], f32, isOutput=False)
    win_w2 = dp("win_w2", [64, 1], f32, isOutput=False)
    win_b2 = dp("win_b2", [1, 1], f32, isOutput=False)
    int_w1 = dp("int_w1", [2 * D, 64], f32, isOutput=False)
    int_b1 = dp("int_b1", [1, 64], f32, isOutput=False)
    int_w2 = dp("int_w2", [64, 1], f32, isOutput=False)
    int_b2 = dp("int_b2", [1, 1], f32, isOutput=False)
    mem_w1 = dp("mem_w1", [2 * D, D], f32, isOutput=False)
    mem_b1 = dp("mem_b1", [1, D], f32, isOutput=False)
    mem_w2 = dp("mem_w2", [D, D], f32, isOutput=False)
    mem_b2 = dp("mem_b2", [1, D], f32, isOutput=False)
    memory_bank = dp("memory_bank", [512, D], f32, isOutput=False)
    up_ws = dp("up_ws", [D, 2 * (D // 2)], f32, isOutput=False)   # per-core strip
    up_bs = dp("up_bs", [1, 2 * (D // 2)], f32, isOutput=False)
    down_ws = dp("down_ws", [D // 2, D], f32, isOutput=False)     # per-core strip
    down_b = dp("down_b", [1, D], f32, isOutput=False)
    n1_g = dp("n1_g", [1, D], f32, isOutput=False)
    n1_b = dp("n1_b", [1, D], f32, isOutput=False)
    n2_g = dp("n2_g", [1, D], f32, isOutput=False)
    n2_b = dp("n2_b", [1, D], f32, isOutput=False)
    rope_sin = dp("rope_sin", [S, D], f32, isOutput=False)
    rope_cos = dp("rope_cos", [S, D], f32, isOutput=False)
    qpoly = dp("qpoly", [1, 4], f32, isOutput=False)
    out_dram = dp("out", [S, D], f32, isOutput=True)

    dbg = {}
    if DEBUG:
        for name, shape in [
            ("dbg_xn", [S, D]), ("dbg_xr", [S, D]), ("dbg_pw", [S, P]),
            ("dbg_inten", [S, 1]), ("dbg_scal", [1, 8]), ("dbg_t0", [S, 1]),
            ("dbg_cnt", [S, 8]), ("dbg_beta", [S, 4]), ("dbg_th", [S, 2]),
            ("dbg_fo", [S, D]), ("dbg_cand", [S, NCAND]),
        ]:
            dbg[name] = dp(name, shape, f32, isOutput=True)

    RG = [list(range(NCORES))]

    with ExitStack() as ctx:
        tc = ctx.enter_context(TileContext(nc))
        # persistent small state (lives for whole kernel)
        pw_ = ctx.enter_context(tc.tile_pool(name="persist", bufs=1))
        # PSUM pools: 6 banks matmul + 2 banks transposes/misc
        pool_mm = ctx.enter_context(tc.tile_pool(name="psumMM", bufs=6, space="PSUM"))
        pool_ps = ctx.enter_context(tc.tile_pool(name="psumT", bufs=2, space="PSUM"))
        pool_dram = ctx.enter_context(tc.tile_pool(name="dramst", bufs=1, space="DRAM"))

        def dma(dst, src):
            nc.sync.dma_start(out=dst, in_=src)

        def bcast_row(pool, src_dram_row, width, name, dtype=f32):
            t = pool.tile([128, width], dtype, name=name)
            dma(t[:], src_dram_row[:].to_broadcast([128, width]))
            return t

        identity = pw_.tile([128, 128], f32, name="identity")
        masks.make_identity(nc, identity[:])
        bc_n = [0]

        def pbcast(pool, dst_ap, src_ap, width, name):
            """broadcast [1,width] sbuf row to [128,width] via a DRAM bounce"""
            bc_n[0] += 1
            st = pool_dram.tile([1, width], f32, name=f"bc{bc_n[0]}_{name}")
            dma(st[:], src_ap)
            dma(dst_ap, st[:].to_broadcast([128, width]))

        def transpose_to(dst_ap, src_ap, name):
            p, f = src_ap.shape[0], src_ap.free_size()
            ps = pool_ps.tile([f, p], f32, name="Tps", tag="Tps",
                              padded_shape=[128, 128])
            nc.tensor.transpose(ps[:f, :p], src_ap, identity[:p, :p])
            nc.vector.tensor_copy(dst_ap, ps[:f, :p])

        ERF_FN = Act.Tanh if SIM_COMPAT else Act.Erf

        def gelu_(pool, ap, name):
            """in-place exact gelu: x * 0.5*(1+erf(x/sqrt(2)))"""
            e = pool.tile(list(ap.shape), f32, name=f"{name}_erf", tag="gelu_e")
            nc.scalar.activation(e[:], ap, ERF_FN, scale=float(1 / np.sqrt(2)))
            nc.vector.tensor_scalar(e[:], e[:], 1.0, 0.5, Alu.add, Alu.mult)
            nc.vector.tensor_tensor(ap, ap, e[:], Alu.mult)

        def silu_(pool, dst_ap, src_ap, name):
            """dst = src * sigmoid(src) (exact identity)"""
            sg = pool.tile(list(src_ap.shape), f32, name=f"{name}_sg", tag="silu_s")
            nc.scalar.activation(sg[:], src_ap, Act.Sigmoid)
            nc.vector.tensor_tensor(dst_ap, src_ap, sg[:], Alu.mult)

        # ---------- persistent tiles ----------
        xg = [pw_.tile([128, D], f32, name=f"xg{g}") for g in range(NG)]
        xn = [pw_.tile([128, D], f32, name=f"xn{g}") for g in range(NG)]
        pwq = [pw_.tile([128, 128], f32, name=f"pwq{g}") for g in range(NG)]
        inten = [pw_.tile([128, 1], f32, name=f"inten{g}") for g in range(NG)]
        kk_b = pw_.tile([128, 1], f32, name="kk_b")
        zq_b = pw_.tile([128, 1], f32, name="zq_b")
        delta_b = pw_.tile([128, 1], f32, name="delta_b")
        invz2_b = pw_.tile([128, 1], f32, name="invz2_b")
        ones_sb = pw_.tile([128, 1], f32, name="ones_sb")
        nc.vector.memset(ones_sb[:], 1.0)
        beta = [(pw_.tile([128, 1], f32, name=f"b1t{g}"),
                 pw_.tile([128, 1], f32, name=f"b2t{g}")) for g in range(NG)]
        rprime = [pw_.tile([128, 1], f32, name=f"rp{g}") for g in range(NG)]
        th = [pw_.tile([128, 1], f32, name=f"th{g}") for g in range(NG)]

        for g in range(NG):
            dma(xg[g][:], x_in[g * 128:(g + 1) * 128, :])

        # =================== preamble (scoped pool) ===================
        with tc.tile_pool(name="preamble", bufs=1) as pp:
            sin_g, cos_g, xr = [], [], []
            for g in range(NG):
                t = pp.tile([128, D], f32, name=f"sin{g}")
                dma(t[:], rope_sin[g * 128:(g + 1) * 128, :])
                sin_g.append(t)
                t = pp.tile([128, D], f32, name=f"cos{g}")
                dma(t[:], rope_cos[g * 128:(g + 1) * 128, :])
                cos_g.append(t)
            n1g_b = bcast_row(pp, n1_g, D, "n1g_b")
            n1b_b = bcast_row(pp, n1_b, D, "n1b_b")

            for g in range(NG):
                mean = pp.tile([128, 1], f32, name=f"mean{g}")
                m2 = pp.tile([128, 1], f32, name=f"m2ln{g}")
                tmp = pp.tile([128, D], f32, name=f"lntmp{g}")
                nc.vector.tensor_reduce(mean[:], xg[g][:], AxX, Alu.add)
                nc.vector.tensor_scalar(mean[:], mean[:], 1.0 / D, None, Alu.mult)
                nc.vector.tensor_scalar(tmp[:], xg[g][:], mean[:], None, Alu.subtract)
                nc.vector.scalar_tensor_tensor(tmp[:], tmp[:], 1.0, tmp[:], Alu.mult,
                                               Alu.mult, accum_out=m2[:])
                nc.vector.tensor_scalar(m2[:], m2[:], 1.0 / D, 1e-5, Alu.mult, Alu.add)
                rstd = pp.tile([128, 1], f32, name=f"rstd{g}")
                nc.scalar.activation(rstd[:], m2[:], Act.Sqrt)
                nc.vector.reciprocal(rstd[:], rstd[:])
                nc.vector.tensor_scalar(xn[g][:], xg[g][:], mean[:], rstd[:],
                                        Alu.subtract, Alu.mult)
                nc.vector.scalar_tensor_tensor(xn[g][:], xn[g][:], 1.0, n1g_b[:],
                                               Alu.mult, Alu.mult)
                nc.vector.tensor_tensor(xn[g][:], xn[g][:], n1b_b[:], Alu.add)
                t_xr = pp.tile([128, D], f32, name=f"xr{g}")
                rot = pp.tile([128, D], f32, name=f"rot{g}")
                ev = lambda a: a.rearrange("p (a two) -> p a two", two=2)[:, :, 0]
                od = lambda a: a.rearrange("p (a two) -> p a two", two=2)[:, :, 1]
                nc.vector.tensor_scalar(ev(rot[:]), od(xn[g][:]), -1.0, None, Alu.mult)
                nc.vector.tensor_copy(od(rot[:]), ev(xn[g][:]))
                nc.vector.tensor_tensor(rot[:], rot[:], sin_g[g][:], Alu.mult)
                nc.vector.scalar_tensor_tensor(t_xr[:], xn[g][:], 1.0, cos_g[g][:],
                                               Alu.mult, Alu.mult)
                nc.vector.tensor_tensor(t_xr[:], t_xr[:], rot[:], Alu.add)
                xr.append(t_xr)

            # ctx = mean over tokens
            ctx_ps = pool_ps.tile([1, D], f32, name="ctx_ps", tag="Tps",
                                  padded_shape=[128, 512])
            for g in range(NG):
                nc.tensor.matmul(ctx_ps[:1, :], ones_sb[:], xr[g][:],
                                 start=(g == 0), stop=(g == NG - 1))
            ctx_row = pp.tile([1, D], f32, name="ctx_row")
            nc.vector.tensor_scalar(ctx_row[:], ctx_ps[:1, :], 1.0 / S, None, Alu.mult)

            xrT = pp.tile([128, 4 * S], f32, name="xrT")
            for g in range(NG):
                for kc in range(4):
                    transpose_to(xrT[:, kc * S + g * 128: kc * S + (g + 1) * 128],
                                 xr[g][:, kc * 128:(kc + 1) * 128], f"xrT{g}{kc}")
            ctxT = pw_.tile([128, 4], f32, name="ctxT")
            for kc in range(4):
                transpose_to(ctxT[:, kc:kc + 1], ctx_row[:, kc * 128:(kc + 1) * 128],
                             f"ctxT{kc}")

            def mlp_head(w1, b1, w2, b2, h1_dim, h2_dim, name):
                w1a = pp.tile([128, 4 * h1_dim], f32, name=f"{name}_w1a")
                w1b = pp.tile([128, 4 * h1_dim], f32, name=f"{name}_w1b")
                for kc in range(4):
                    dma(w1a[:, kc * h1_dim:(kc + 1) * h1_dim],
                        w1[kc * 128:(kc + 1) * 128, :])
                    dma(w1b[:, kc * h1_dim:(kc + 1) * h1_dim],
                        w1[D + kc * 128: D + (kc + 1) * 128, :])
                b1_b = bcast_row(pp, b1, h1_dim, f"{name}_b1b")
                w2_sb = pp.tile([h1_dim, h2_dim], f32, name=f"{name}_w2sb")
                dma(w2_sb[:], w2[:])
                b2_b = bcast_row(pp, b2, h2_dim, f"{name}_b2b")
                v1_ps = pool_ps.tile([1, h1_dim], f32, name="v1ps", tag="Tps",
                                     padded_shape=[128, 128])
                for kc in range(4):
                    nc.tensor.matmul(v1_ps[:1, :], ctxT[:, kc:kc + 1],
                                     w1b[:, kc * h1_dim:(kc + 1) * h1_dim],
                                     start=(kc == 0), stop=(kc == 3))
                v1 = pp.tile([1, h1_dim], f32, name=f"{name}_v1")
                nc.vector.tensor_copy(v1[:], v1_ps[:1, :])
                v1_b = pp.tile([128, h1_dim], f32, name=f"{name}_v1b")
                pbcast(pp, v1_b[:], v1[:], h1_dim, f"{name}v1")
                outs = []
                for g in range(NG):
                    h1_ps = pool_ps.tile([128, h1_dim], f32, name="h1ps", tag="Tps",
                                         padded_shape=[128, 128])
                    for kc in range(4):
                        nc.tensor.matmul(
                            h1_ps[:], xrT[:, kc * S + g * 128: kc * S + (g + 1) * 128],
                            w1a[:, kc * h1_dim:(kc + 1) * h1_dim],
                            start=(kc == 0), stop=(kc == 3))
                    h1 = pp.tile([128, h1_dim], f32, name=f"{name}_h1_{g}")
                    nc.vector.tensor_tensor(h1[:], h1_ps[:], v1_b[:], Alu.add)
                    nc.vector.tensor_tensor(h1[:], h1[:], b1_b[:], Alu.add)
                    gelu_(pp, h1[:], f"{name}g{g}")
                    h1T = pp.tile([h1_dim, 128], f32, name=f"{name}_h1T_{g}")
                    transpose_to(h1T[:], h1[:], f"{name}h1T{g}")
                    h2_ps = pool_ps.tile([128, h2_dim], f32, name="h2ps", tag="Tps",
                                         padded_shape=[128, 128])
                    nc.tensor.matmul(h2_ps[:], h1T[:], w2_sb[:], start=True, stop=True)
                    h2 = pp.tile([128, h2_dim], f32, name=f"{name}_h2_{g}")
                    nc.vector.tensor_tensor(h2[:], h2_ps[:], b2_b[:], Alu.add)
                    outs.append(h2)
                return outs

            sel_h2 = mlp_head(sel_w1, sel_b1, sel_w2, sel_b2, 2 * P, P, "sel")
            int_h2 = mlp_head(int_w1, int_b1, int_w2, int_b2, 64, 1, "intm")

            for g in range(NG):
                t_pw = pp.tile([128, P], f32, name=f"pwsm{g}")
                mx = pp.tile([128, 1], f32, name=f"selmx{g}")
                nc.vector.tensor_reduce(mx[:], sel_h2[g][:], AxX, Alu.max)
                nc.vector.tensor_scalar(sel_h2[g][:], sel_h2[g][:], mx[:], None,
                                        Alu.subtract)
                nc.scalar.activation(sel_h2[g][:], sel_h2[g][:], Act.Exp)
                sm = pp.tile([128, 1], f32, name=f"selsm{g}")
                nc.vector.tensor_reduce(sm[:], sel_h2[g][:], AxX, Alu.add)
                rs = pp.tile([128, 1], f32, name=f"selrs{g}")
                nc.vector.reciprocal(rs[:], sm[:])
                nc.vector.tensor_scalar(t_pw[:], sel_h2[g][:], rs[:], None, Alu.mult)
                nc.scalar.activation(inten[g][:], int_h2[g][:], Act.Sigmoid)
                # pw^T replicated into the 4 PE row-quadrants (via SBUF->SBUF DMA)
                pwt0 = pp.tile([P, 128], f32, name=f"pwt0_{g}")
                transpose_to(pwt0[:], t_pw[:], f"pwT{g}")
                for q in range(4):
                    dma(pwq[g][32 * q:32 * q + 16, :], pwt0[:])
                if DEBUG:
                    dma(dbg["dbg_pw"][g * 128:(g + 1) * 128, :], t_pw[:])


            # window scalar -> kk, z, delta
            winw1_sb = pp.tile([128, 4 * 64], f32, name="winw1_sb")
            for kc in range(4):
                dma(winw1_sb[:, kc * 64:(kc + 1) * 64],
                    win_w1[kc * 128:(kc + 1) * 128, :])
            wh1_ps = pool_ps.tile([1, 64], f32, name="wh1ps", tag="Tps",
                                  padded_shape=[128, 128])
            for kc in range(4):
                nc.tensor.matmul(wh1_ps[:1, :], ctxT[:, kc:kc + 1],
                                 winw1_sb[:, kc * 64:(kc + 1) * 64],
                                 start=(kc == 0), stop=(kc == 3))
            wh1 = pp.tile([1, 64], f32, name="wh1")
            wb1_sb = pp.tile([1, 64], f32, name="wb1_sb")
            dma(wb1_sb[:], win_b1[:])
            nc.vector.tensor_tensor(wh1[:], wh1_ps[:1, :], wb1_sb[:], Alu.add)
            gelu_(pp, wh1[:], "wh1g")
            wh1T = pp.tile([64, 1], f32, name="wh1T")
            transpose_to(wh1T[:], wh1[:], "wh1T")
            winw2_sb = pp.tile([64, 1], f32, name="winw2_sb")
            dma(winw2_sb[:], win_w2[:])
            win_ps = pool_ps.tile([1, 1], f32, name="winps", tag="Tps",
                                  padded_shape=[128, 128])
            nc.tensor.matmul(win_ps[:1, :1], wh1T[:], winw2_sb[:], start=True,
                             stop=True)
            winv = pp.tile([1, 1], f32, name="winv")
            wb2_sb = pp.tile([1, 1], f32, name="wb2_sb")
            dma(wb2_sb[:], win_b2[:])
            nc.vector.tensor_tensor(winv[:], win_ps[:1, :1], wb2_sb[:], Alu.add)
            nc.scalar.activation(winv[:], winv[:], Act.Sigmoid)
            nc.vector.tensor_scalar(winv[:], winv[:], float(MAX_SEQ - 256), 256.0,
                                    Alu.mult, Alu.add)
            kkf = pp.tile([1, 1], f32, name="kkf")
            nc.vector.tensor_scalar(kkf[:], winv[:], 0.1 / MAX_SEQ * DD, None,
                                    Alu.mult)
            # floor() robust to the f32->i32 convert rounding mode
            ki = pp.tile([1, 1], dt.int32, name="ki")
            nc.vector.tensor_copy(ki[:], kkf[:])
            kf2 = pp.tile([1, 1], f32, name="kf2")
            nc.vector.tensor_copy(kf2[:], ki[:])
            kgt = pp.tile([1, 1], f32, name="kgt")
            nc.vector.tensor_tensor(kgt[:], kf2[:], kkf[:], Alu.is_gt)
            nc.vector.tensor_tensor(kkf[:], kf2[:], kgt[:], Alu.subtract)
            nc.vector.tensor_scalar(kkf[:], kkf[:], 1.0, None, Alu.max)

            qp = pp.tile([1, 4], f32, name="qp")
            dma(qp[:], qpoly[:])
            u = pp.tile([1, 1], f32, name="qu")
            nc.vector.tensor_scalar(u[:], kkf[:], 1.0 / DD, None, Alu.mult)
            nc.scalar.activation(u[:], u[:], Act.Ln)
            zq = pp.tile([1, 1], f32, name="zq")
            nc.vector.tensor_scalar(zq[:], qp[:, 0:1], u[:], qp[:, 1:2], Alu.mult,
                                    Alu.add)
            nc.vector.tensor_scalar(zq[:], zq[:], u[:], qp[:, 2:3], Alu.mult, Alu.add)
            nc.vector.tensor_scalar(zq[:], zq[:], u[:], qp[:, 3:4], Alu.mult, Alu.add)
            phi = pp.tile([1, 1], f32, name="phi")
            nc.vector.scalar_tensor_tensor(phi[:], zq[:], -0.5, zq[:], Alu.mult,
                                           Alu.mult)
            nc.scalar.activation(phi[:], phi[:], Act.Exp)
            nc.vector.tensor_scalar(phi[:], phi[:], float(1.0 / np.sqrt(2 * np.pi)),
                                    None, Alu.mult)
            dens = pp.tile([1, 1], f32, name="dens")
            nc.vector.scalar_tensor_tensor(dens[:], phi[:], float(2.0 * DD), zq[:],
                                           Alu.mult, Alu.mult)
            delta = pp.tile([1, 1], f32, name="delta")
            nc.vector.reciprocal(delta[:], dens[:])
            nc.vector.tensor_scalar(delta[:], delta[:], BAND, None, Alu.mult)
            pbcast(pp, kk_b[:], kkf[:], 1, "kk")
            pbcast(pp, zq_b[:], zq[:], 1, "zq")
            pbcast(pp, delta_b[:], delta[:], 1, "delta")
            nc.vector.scalar_tensor_tensor(invz2_b[:], zq_b[:], 1.0, zq_b[:],
                                           Alu.mult, Alu.mult)
            nc.vector.reciprocal(invz2_b[:], invz2_b[:])
            if DEBUG:
                dma(dbg["dbg_scal"][:, 0:1], kkf[:])
                dma(dbg["dbg_scal"][:, 1:2], winv[:])
                dma(dbg["dbg_scal"][:, 2:3], zq[:])
                dma(dbg["dbg_scal"][:, 3:4], delta[:])

            if DEBUG:
                for g in range(NG):
                    dma(dbg["dbg_xn"][g * 128:(g + 1) * 128, :], xn[g][:])
                    dma(dbg["dbg_xr"][g * 128:(g + 1) * 128, :], xr[g][:])
                    dma(dbg["dbg_inten"][g * 128:(g + 1) * 128, :], inten[g][:])

        if STAGE < 2:
            for g in range(NG):
                dma(out_dram[g * 128:(g + 1) * 128, :], xg[g][:])
            return nc

        # ====== helper: quad-packed F rematerialization (stream patterns) ======
        def flow_quad(g, consume, pat_pool, swlist=None):
            """consume(c, psum_ap) for each 512-chunk c (i_loc = c) of group g.

            4 concurrent K=16 matmuls on the PE row-quadrants; super-window sw
            covers chunks [16*sw, 16*sw+16).
            """
            for sw in (swlist if swlist is not None else range(4)):
                patq = pat_pool.tile([128, 2048], f32, name="patq", tag="patq",
                                     bufs=3)
                for q in range(4):
                    w = sw * 4 + q
                    dma(patq[32 * q:32 * q + 16, :],
                        pat_sl[:, w * 2048:(w + 1) * 2048])
                for m in range(4):
                    for q in range(4):
                        c = (sw * 4 + q) * 4 + m
                        ps = pool_mm.tile([128, 512], f32, name="Fps", tag="Fps")
                        nc.tensor.matmul(
                            ps[:], mmc(pwq[g][32 * q:32 * q + 16, :]),
                            mmc(patq[32 * q:32 * q + 16, m * 512:(m + 1) * 512]),
                            start=True, stop=True, tile_position=(32 * q, 0))
                        consume(c, ps)

        t0_stage = [pool_dram.tile([128, 1], f32, name=f"t0_stage{g}")
                    for g in range(NG)]
        t0_out = [pool_dram.tile([128, 1], f32, name=f"t0_out{g}",
                                 addr_space="Shared") for g in range(NG)]
        cnt_stage = [pool_dram.tile([128, NL], f32, name=f"cnt_stage{g}")
                     for g in range(NG)]
        cnt_out = [pool_dram.tile([128, NL], f32, name=f"cnt_out{g}",
                                  addr_space="Shared") for g in range(NG)]
        cand_stage = [pool_dram.tile([128, NCAND], f32, name=f"cand_stage{g}")
                      for g in range(NG)]
        cand_out = [pool_dram.tile([NCORES, 128, NCAND], f32, name=f"cand_out{g}",
                                   addr_space="Shared") for g in range(NG)]

        tlad_all = []
        # =============== P1 + selection ladder (scoped pool) ===============
        with tc.tile_pool(name="selpool", bufs=1) as sp:
            A_bf = sp.tile([128, NG * FREE], bf16, name="A_bf")
            scr_d = sp.tile([128, LAD_DVE // 2], bf16, name="scr_d")
            scr_a = sp.tile([128, LAD_ACT], bf16, name="scr_a")

            for g in range(NG):
                def consume_p1(c, ps, g=g):
                    dst = A_bf[:, g * FREE + c * 512: g * FREE + (c + 1) * 512]
                    nc.scalar.activation(dst, ps[:], Act.Abs,
                                         scale=inten[g][:])
                flow_quad(g, consume_p1, sp)


            # moments + Newton per group (both on DVE; groups independent)
            for g in range(NG):
                Ag = A_bf[:, g * FREE:(g + 1) * FREE]
                m4 = sp.tile([128, 4], f32, name=f"m4_{g}")
                for q in range(4):
                    nc.vector.scalar_tensor_tensor(
                        scr_d[:, :8192], Ag[:, q * 8192:(q + 1) * 8192], 1.0,
                        Ag[:, q * 8192:(q + 1) * 8192], Alu.mult, Alu.mult,
                        accum_out=m4[:, q:q + 1])
                m2a = sp.tile([128, 1], f32, name=f"m2a{g}")
                nc.vector.tensor_reduce(m2a[:], m4[:], AxX, Alu.add)
                sig = sp.tile([128, 1], f32, name=f"sig{g}")
                nc.vector.tensor_scalar(sig[:], m2a[:], 1.0 / FREE, None, Alu.mult)
                nc.scalar.activation(sig[:], sig[:], Act.Sqrt)
                t0 = sp.tile([128, 1], f32, name=f"t0{g}")
                nc.vector.tensor_tensor(t0[:], sig[:], zq_b[:], Alu.mult)

                # Newton on a 1/4 chunk-contiguous sample (8192 elems)
                Asmp = Ag.rearrange("p (a b c) -> p a b c", b=4, c=512)[:, :, 0, :]
                cs = sp.tile([128, 1], f32, name=f"cs{g}")
                lnr = sp.tile([128, 1], f32, name=f"lnr{g}")
                ktgt = sp.tile([128, 1], f32, name=f"ktgt{g}")
                nc.vector.tensor_scalar(ktgt[:], kk_b[:], 1.0 / 32.0, None, Alu.mult)
                rtg = sp.tile([128, 1], f32, name=f"rtg{g}")
                nc.vector.reciprocal(rtg[:], ktgt[:])
                scr_s = scr_d[:, :8192].rearrange("p (a c) -> p a c", c=512)
                for it in range(4):
                    nc.vector.tensor_scalar(scr_s, Asmp, t0[:],
                                            None, Alu.is_ge, Alu.add, accum_out=cs[:])
                    nc.vector.tensor_scalar(cs[:], cs[:], 1.0, None, Alu.max)
                    nc.vector.tensor_tensor(lnr[:], cs[:], rtg[:], Alu.mult)
                    nc.vector.tensor_scalar(lnr[:], lnr[:], 0.1, 10.0, Alu.max,
                                            Alu.min)
                    nc.scalar.activation(lnr[:], lnr[:], Act.Ln)
                    nc.vector.tensor_tensor(lnr[:], lnr[:], invz2_b[:], Alu.mult)
                    nc.scalar.activation(lnr[:], lnr[:], Act.Exp)
                    nc.vector.tensor_tensor(t0[:], t0[:], lnr[:], Alu.mult)
                dma(t0_stage[g][:], t0[:])
                # harmonize t0 across cores per group (overlaps next group's
                # moments/Newton; ladders must be identical everywhere)
                nc.gpsimd.collective_compute(
                    "AllReduce", Alu.add, replica_groups=RG,
                    ins=[t0_stage[g][:]], outs=[t0_out[g][:]])

            for g in range(NG):
                Ag = A_bf[:, g * FREE:(g + 1) * FREE]
                t0 = sp.tile([128, 1], f32, name=f"t0h{g}")
                dma(t0[:], t0_out[g][:])
                nc.vector.tensor_scalar(t0[:], t0[:], 1.0 / NCORES, None, Alu.mult)
                if DEBUG:
                    dma(dbg["dbg_t0"][g * 128:(g + 1) * 128, :], t0[:])

                tl = pw_.tile([128, NL], f32, name=f"tlad{g}")
                tl_bf = sp.tile([128, NL], bf16, name=f"tladbf{g}")
                fac = sp.tile([128, 1], f32, name=f"fac{g}")
                for j in range(NL):
                    nc.vector.tensor_scalar(fac[:], delta_b[:], float(j - NL // 2),
                                            None, Alu.mult)
                    nc.scalar.activation(fac[:], fac[:], Act.Exp)
                    nc.vector.tensor_tensor(tl[:, j:j + 1], t0[:], fac[:], Alu.mult)
                nc.vector.tensor_copy(tl_bf[:], tl[:])
                nc.vector.tensor_copy(tl[:], tl_bf[:])
                tlad_all.append(tl)

                # beta-midpoint thresholds for the ACT Sign counts (tie-free)
                nbeta = sp.tile([128, NL], f32, name=f"nbeta{g}")
                pvl = sp.tile([128, NL], f32, name=f"pvl{g}")
                pvl_bf = sp.tile([128, NL], bf16, name=f"pvlbf{g}")
                nc.vector.tensor_scalar(pvl[:], tl[:], float(1.0 - 2.0 ** -8), None,
                                        Alu.mult)
                nc.vector.tensor_copy(pvl_bf[:], pvl[:])
                nc.vector.tensor_copy(pvl[:], pvl_bf[:])
                nc.vector.tensor_tensor(pvl[:], pvl[:], tl[:], Alu.add)
                nc.vector.tensor_scalar(nbeta[:], pvl[:], -0.5, None, Alu.mult)

                cl = sp.tile([128, NL], f32, name=f"cl{g}")
                HD = LAD_DVE // 2
                for j in range(NL):
                    # per-rung accumulator tile so rungs pipeline (no WAR chain)
                    acc4 = sp.tile([128, 4], f32, name=f"acc4_{g}_{j}",
                                   tag="acc4", bufs=4)
                    # DVE: 2 segments (is_ge, ties counted)
                    for s2 in range(2):
                        nc.vector.tensor_scalar(
                            scr_d[:, :HD], Ag[:, s2 * HD:(s2 + 1) * HD],
                            tl[:, j:j + 1], None, Alu.is_ge, Alu.add,
                            accum_out=acc4[:, s2:s2 + 1])
                    # ACT: 2 segments (Sign at beta midpoint, tie-free)
                    for s2 in range(2):
                        lo = LAD_DVE + s2 * LAD_ACT
                        nc.scalar.activation(
                            scr_a[:, :LAD_ACT], Ag[:, lo:lo + LAD_ACT], Act.Sign,
                            bias=nbeta[:, j:j + 1],
                            accum_out=acc4[:, 2 + s2:3 + s2])
                    nc.vector.tensor_tensor(cl[:, j:j + 1], acc4[:, 0:1],
                                            acc4[:, 1:2], Alu.add)
                    csum = sp.tile([128, 1], f32, name=f"csum{g}_{j}", tag="csum",
                                   bufs=4)
                    nc.vector.tensor_tensor(csum[:], acc4[:, 2:3], acc4[:, 3:4],
                                            Alu.add)
                    nc.vector.tensor_scalar(csum[:], csum[:],
                                            float(2 * LAD_ACT), 0.5, Alu.add,
                                            Alu.mult)
                    nc.vector.tensor_tensor(cl[:, j:j + 1], cl[:, j:j + 1], csum[:],
                                            Alu.add)
                dma(cnt_stage[g][:], cl[:])
                # per-group count AllReduce: g0's reduce hides under g1's ladder
                nc.gpsimd.collective_compute(
                    "AllReduce", Alu.add, replica_groups=RG,
                    ins=[cnt_stage[g][:]], outs=[cnt_out[g][:]])

        tailP = ctx.enter_context(tc.tile_pool(name="tailP", bufs=1))
        # ====== P3 pool opens early: quarter (g0,sw0) remat overlaps bracket ======
        p3ctx = tc.tile_pool(name="p3pool", bufs=1)
        xp = p3ctx.__enter__()
        Xq00 = xp.tile([128, 8192], f32, name="Xq", tag="Xq", bufs=2)

        def consume_pre(c, ps):
            nc.scalar.activation(Xq00[:, c * 512:(c + 1) * 512],
                                 ps[:], Act.Abs, scale=inten[0][:])
        flow_quad(0, consume_pre, xp, swlist=[0])

        # bracket selection (small persistent tiles)
        with tc.tile_pool(name="bracket", bufs=1) as bp:
            for g in range(NG):
                cl = bp.tile([128, NL], f32, name=f"clg{g}")
                dma(cl[:], cnt_out[g][:])
                if DEBUG:
                    dma(dbg["dbg_cnt"][g * 128:(g + 1) * 128, 0:NL], cl[:])
                ge = bp.tile([128, NL], f32, name=f"ge{g}")
                nc.vector.tensor_scalar(ge[:], cl[:], kk_b[:], None, Alu.is_ge)
                sel = bp.tile([128, NL - 1], f32, name=f"sel{g}")
                nc.vector.tensor_scalar(sel[:], ge[:, 1:NL], -1.0, 1.0, Alu.mult,
                                        Alu.add)
                nc.vector.tensor_tensor(sel[:], sel[:], ge[:, 0:NL - 1], Alu.mult)
                t1 = bp.tile([128, 1], f32, name=f"t1_{g}")
                t2 = bp.tile([128, 1], f32, name=f"t2_{g}")
                c2 = bp.tile([128, 1], f32, name=f"c2_{g}")
                stmp = bp.tile([128, NL - 1], f32, name=f"stmp{g}")
                tl = tlad_all[g]
                nc.vector.tensor_tensor(stmp[:], sel[:], tl[:, 0:NL - 1], Alu.mult)
                nc.vector.tensor_reduce(t1[:], stmp[:], AxX, Alu.add)
                nc.vector.tensor_tensor(stmp[:], sel[:], tl[:, 1:NL], Alu.mult)
                nc.vector.tensor_reduce(t2[:], stmp[:], AxX, Alu.add)
                nc.vector.tensor_tensor(stmp[:], sel[:], cl[:, 1:NL], Alu.mult)
                nc.vector.tensor_reduce(c2[:], stmp[:], AxX, Alu.add)
                # exact fp32 count-boundary of a bf16 threshold t:
                # beta = (t + prev16(t))/2 with prev16(t) = bf16RTN(t*(1-2^-8))
                pv = bp.tile([128, 2], f32, name=f"pv{g}")
                pv_bf = bp.tile([128, 2], bf16, name=f"pvbf{g}")
                nc.vector.tensor_scalar(pv[:, 0:1], t1[:],
                                        float(1.0 - 2.0 ** -8), None, Alu.mult)
                nc.vector.tensor_scalar(pv[:, 1:2], t2[:],
                                        float(1.0 - 2.0 ** -8), None, Alu.mult)
                nc.vector.tensor_copy(pv_bf[:], pv[:])
                nc.vector.tensor_copy(pv[:], pv_bf[:])
                nc.vector.tensor_tensor(pv[:, 0:1], pv[:, 0:1], t1[:], Alu.add)
                nc.vector.tensor_tensor(pv[:, 1:2], pv[:, 1:2], t2[:], Alu.add)
                nc.vector.tensor_scalar(beta[g][0][:], pv[:, 0:1], 0.5, None,
                                        Alu.mult)
                nc.vector.tensor_scalar(beta[g][1][:], pv[:, 1:2], 0.5, None,
                                        Alu.mult)
                nc.vector.scalar_tensor_tensor(rprime[g][:], c2[:], -1.0, kk_b[:],
                                               Alu.mult, Alu.add)
                if DEBUG:
                    dma(dbg["dbg_beta"][g * 128:(g + 1) * 128, 0:1], beta[g][0][:])
                    dma(dbg["dbg_beta"][g * 128:(g + 1) * 128, 1:2], beta[g][1][:])
                    dma(dbg["dbg_beta"][g * 128:(g + 1) * 128, 2:3], c2[:])
                    dma(dbg["dbg_beta"][g * 128:(g + 1) * 128, 3:4], rprime[g][:])

        # ====== P3: band extraction (top-2 per 64-chunk, quarter pipeline) ======
        if True:
            for g in range(NG):
                b1t, b2t = beta[g]
                cand = xp.tile([128, NCAND], f32, name="cand", tag="cand")
                for sw in range(4):
                    if g == 0 and sw == 0:
                        Xq = Xq00
                    else:
                        Xq = xp.tile([128, 8192], f32, name="Xq", tag="Xq",
                                     bufs=2)
                    Zq = xp.tile([128, 8192], f32, name="Zq", tag="Zq", bufs=2)

                    def consume_p3(c, ps, g=g, Xq=Xq, sw=sw):
                        cc = c - sw * 16
                        nc.scalar.activation(Xq[:, cc * 512:(cc + 1) * 512],
                                             ps[:], Act.Abs, scale=inten[g][:])
                    if not (g == 0 and sw == 0):
                        flow_quad(g, consume_p3, xp, swlist=[sw])
                    nc.vector.scalar_tensor_tensor(Zq[:], Xq[:], b2t[:], Xq[:],
                                                   Alu.is_lt, Alu.mult)
                    ch = lambda a: a.rearrange("p (c e) -> p c e", e=64)
                    NQ = 128  # 64-chunks per quarter
                    L1 = xp.tile([128, NQ], f32, name="L1", tag="L1", bufs=2)
                    nc.vector.tensor_reduce(L1[:], ch(Zq[:]), AxX, Alu.max)
                    L1b = L1[:].rearrange("p (c one) -> p c one", one=1).to_broadcast(
                        [128, NQ, 64])
                    nc.vector.tensor_tensor(ch(Xq[:]), ch(Zq[:]), L1b, Alu.is_lt)
                    nc.vector.tensor_tensor(Zq[:], Zq[:], Xq[:], Alu.mult)
                    L2 = xp.tile([128, NQ], f32, name="L2", tag="L2", bufs=2)
                    nc.vector.tensor_reduce(L2[:], ch(Zq[:]), AxX, Alu.max)
                    nc.vector.scalar_tensor_tensor(L1[:], L1[:], b1t[:], L1[:],
                                                   Alu.is_ge, Alu.mult)
                    nc.vector.scalar_tensor_tensor(L2[:], L2[:], b1t[:], L2[:],
                                                   Alu.is_ge, Alu.mult)
                    nc.vector.tensor_copy(cand[:, sw * NQ:(sw + 1) * NQ], L1[:])
                    nc.vector.tensor_copy(cand[:, 512 + sw * NQ:512 + (sw + 1) * NQ],
                                          L2[:])
                dma(cand_stage[g][:], cand[:])
                nc.gpsimd.collective_compute(
                    "AllGather", Alu.bypass, replica_groups=RG,
                    ins=[cand_stage[g][:]], outs=[cand_out[g][:]])
            p3ctx.__exit__(None, None, None)

        # ========= exact threshold: replicated bisection (DVE+ACT split), =========
        # ========= interleaved with P4 so P4-g0 weaves into bisect-g1     =========
        GW = NCORES * NCAND
        fo_stage = [pool_dram.tile([128, ISLICE], f32, name=f"fo_stage{g}")
                    for g in range(NG)]
        fo_out = [pool_dram.tile([NCORES, 128, ISLICE], f32, name=f"fo_out{g}",
                                 addr_space="Shared") for g in range(NG)]
        fo_full = [tailP.tile([128, D], f32, name=f"fo_full{g}") for g in range(NG)]
        with tc.tile_pool(name="bisect", bufs=1) as gp, \
                tc.tile_pool(name="p4pool", bufs=1) as fp:
            XI = []
            for g in range(NG):
                t = fp.tile([128, D], f32, name=f"XI{g}")
                nc.vector.tensor_scalar(t[:], xn[g][:], inten[g][:], None, Alu.mult)
                XI.append(t)
            T = {}
            for g in range(NG):
                T[g] = dict(
                    G=gp.tile([128, GW], f32, name=f"Gc{g}"),
                    gsc=gp.tile([128, GW], f32, name=f"gsc{g}"),
                    lo=gp.tile([128, 1], f32, name=f"lo{g}"),
                    hi=gp.tile([128, 1], f32, name=f"hi{g}"),
                    mid=gp.tile([128, 1], f32, name=f"mid{g}"),
                    nmid=gp.tile([128, 1], f32, name=f"nmid{g}"),
                    cm=gp.tile([128, 1], f32, name=f"cm{g}"),
                    cma=gp.tile([128, 1], f32, name=f"cma{g}"),
                    sl=gp.tile([128, 1], f32, name=f"sl{g}"),
                    dm=gp.tile([128, 1], f32, name=f"dm{g}"),
                    dh=gp.tile([128, 1], f32, name=f"dh{g}"),
                )
                t = T[g]
                for cidx in range(NCORES):
                    dma(t["G"][:, cidx * NCAND:(cidx + 1) * NCAND],
                        cand_out[g][cidx, :, :])
                if DEBUG and g == 0:
                    dma(dbg["dbg_cand"][0:128, :], t["G"][:, 0:NCAND])
                nc.vector.tensor_copy(t["lo"][:], beta[g][0][:])
                nc.vector.tensor_copy(t["hi"][:], beta[g][1][:])

                G, gsc = t["G"], t["gsc"]
                lo, hi, mid, nmid = t["lo"], t["hi"], t["mid"], t["nmid"]
                cm, cma, sl, dm, dh = (t["cm"], t["cma"], t["sl"], t["dm"],
                                       t["dh"])
                for _ in range(N_BISECT):
                    nc.vector.tensor_tensor(mid[:], lo[:], hi[:], Alu.add)
                    nc.vector.tensor_scalar(mid[:], mid[:], 0.5, None, Alu.mult)
                    nc.vector.tensor_scalar(nmid[:], mid[:], -1.0, None, Alu.mult)
                    # DVE slice + ACT slice, concurrently
                    nc.vector.tensor_scalar(gsc[:, :BIS_DVE], G[:, :BIS_DVE],
                                            mid[:], None, Alu.is_ge, Alu.add,
                                            accum_out=cm[:])
                    nc.scalar.activation(gsc[:, BIS_DVE:], G[:, BIS_DVE:],
                                         Act.Sign, bias=nmid[:],
                                         accum_out=cma[:])
                    nc.vector.tensor_scalar(cma[:], cma[:], float(GW - BIS_DVE),
                                            0.5, Alu.add, Alu.mult)
                    nc.vector.tensor_tensor(cm[:], cm[:], cma[:], Alu.add)
                    # fused interval update: sl = (cm>=r'); lo += (mid-lo)*sl;
                    # hi = mid + (hi-mid)*sl
                    nc.vector.tensor_scalar(sl[:], cm[:], rprime[g][:], None,
                                            Alu.is_ge)
                    nc.vector.tensor_tensor(dm[:], mid[:], lo[:], Alu.subtract)
                    nc.vector.scalar_tensor_tensor(lo[:], dm[:], sl[:], lo[:],
                                                   Alu.mult, Alu.add)
                    nc.vector.tensor_tensor(dh[:], hi[:], mid[:], Alu.subtract)
                    nc.vector.scalar_tensor_tensor(hi[:], dh[:], sl[:], mid[:],
                                                   Alu.mult, Alu.add)

                # cHI = count(G >= hi) (exact, DVE)
                cHI = gp.tile([128, 1], f32, name=f"cHI{g}")
                nc.vector.tensor_scalar(gsc[:], G[:], hi[:], None, Alu.is_ge,
                                        Alu.add, accum_out=cHI[:])
                # window-mask G below hi only; below-lo values are harmless
                # for the count-based rank select (always smaller than window)
                nc.vector.scalar_tensor_tensor(G[:], G[:], hi[:], G[:], Alu.is_lt,
                                               Alu.mult)
                W8 = gp.tile([128, 8], f32, name=f"W8{g}")
                nc.vector.max(out=W8[:], in_=G[:])
                # idx = rprime - cHI; th = idx-th largest of W8 (duplicate-safe:
                # th = max{v in W8 : count(W8 >= v) >= idx}), fallback hi if
                # idx <= 0
                idx = gp.tile([128, 1], f32, name=f"idx{g}")
                nc.vector.scalar_tensor_tensor(idx[:], cHI[:], -1.0, rprime[g][:],
                                               Alu.mult, Alu.add)
                c8 = gp.tile([128, 8], f32, name=f"c8{g}")
                scr8 = gp.tile([128, 8], f32, name=f"scr8{g}")
                for r in range(8):
                    nc.vector.tensor_scalar(scr8[:], W8[:], W8[:, r:r + 1], None,
                                            Alu.is_ge, Alu.add,
                                            accum_out=c8[:, r:r + 1])
                nc.vector.tensor_scalar(c8[:], c8[:], idx[:], None, Alu.is_ge)
                nc.vector.tensor_tensor(c8[:], c8[:], W8[:], Alu.mult)
                vsel = gp.tile([128, 1], f32, name=f"vsel{g}")
                nc.vector.tensor_reduce(vsel[:], c8[:], AxX, Alu.max)
                acc = gp.tile([128, 1], f32, name=f"thacc{g}")
                msk = gp.tile([128, 1], f32, name=f"thmsk{g}")
                nc.vector.tensor_scalar(msk[:], idx[:], 0.5, None, Alu.is_le)
                nc.vector.tensor_tensor(acc[:], msk[:], hi[:], Alu.mult)
                nc.vector.tensor_scalar(msk[:], msk[:], -1.0, 1.0, Alu.mult,
                                        Alu.add)
                nc.vector.tensor_tensor(msk[:], msk[:], vsel[:], Alu.mult)
                nc.vector.tensor_tensor(acc[:], acc[:], msk[:], Alu.add)
                nc.vector.tensor_copy(th[g][:], acc[:])
                if DEBUG:
                    dma(dbg["dbg_th"][g * 128:(g + 1) * 128, 0:1], th[g][:])
                    dma(dbg["dbg_th"][g * 128:(g + 1) * 128, 1:2], rprime[g][:])

            # ---- P4: final masked matvec (after both groups' thresholds)
            if STAGE >= 3:
                for g in range(NG):
                    FO = fp.tile([128, ISLICE], f32, name=f"FO{g}")

                    def consume_p4(c, ps, g=g, FO=FO):
                        At = fp.tile([128, 512], f32, name="At", tag="At", bufs=6)
                        FM = fp.tile([128, 512], f32, name="FM", tag="FM", bufs=6)
                        nc.scalar.activation(At[:], ps[:], Act.Abs,
                                             scale=inten[g][:])
                        nc.vector.scalar_tensor_tensor(FM[:], At[:], th[g][:],
                                                       ps[:], Alu.is_ge, Alu.mult)
                        nc.vector.scalar_tensor_tensor(FM[:], FM[:], 1.0, XI[g][:],
                                                       Alu.mult, Alu.mult,
                                                       accum_out=FO[:, c:c + 1])
                    flow_quad(g, consume_p4, fp)
                    dma(fo_stage[g][:], FO[:])
                    nc.gpsimd.collective_compute(
                        "AllGather", Alu.bypass, replica_groups=RG,
                        ins=[fo_stage[g][:]], outs=[fo_out[g][:]])

        if STAGE < 3:
            for g in range(NG):
                dma(out_dram[g * 128:(g + 1) * 128, :], xg[g][:])
            return nc

        # =============== tail ===============
        co = [tailP.tile([128, D], f32, name=f"co{g}") for g in range(NG)]
        with tc.tile_pool(name="tail1", bufs=1) as tp:
            n2g_b = bcast_row(tp, n2_g, D, "n2g_b")
            n2b_b = bcast_row(tp, n2_b, D, "n2b_b")
            for g in range(NG):
                for cidx in range(NCORES):
                    dma(fo_full[g][:, cidx * ISLICE:(cidx + 1) * ISLICE],
                        fo_out[g][cidx, :, :])
                if DEBUG:
                    dma(dbg["dbg_fo"][g * 128:(g + 1) * 128, :], fo_full[g][:])
                nc.vector.tensor_tensor(co[g][:], xg[g][:], fo_full[g][:], Alu.add)
                mean = tp.tile([128, 1], f32, name=f"mean2{g}")
                m2 = tp.tile([128, 1], f32, name=f"m2ln2{g}")
                tmp = tp.tile([128, D], f32, name=f"ln2tmp{g}", tag="tmp")
                nc.vector.tensor_reduce(mean[:], co[g][:], AxX, Alu.add)
                nc.vector.tensor_scalar(mean[:], mean[:], 1.0 / D, None, Alu.mult)
                nc.vector.tensor_scalar(tmp[:], co[g][:], mean[:], None,
                                        Alu.subtract)
                nc.vector.scalar_tensor_tensor(tmp[:], tmp[:], 1.0, tmp[:], Alu.mult,
                                               Alu.mult, accum_out=m2[:])
                nc.vector.tensor_scalar(m2[:], m2[:], 1.0 / D, 1e-5, Alu.mult,
                                        Alu.add)
                rstd = tp.tile([128, 1], f32, name=f"rstd2{g}")
                nc.scalar.activation(rstd[:], m2[:], Act.Sqrt)
                nc.vector.reciprocal(rstd[:], rstd[:])
                nc.vector.tensor_scalar(co[g][:], co[g][:], mean[:], rstd[:],
                                        Alu.subtract, Alu.mult)
                nc.vector.scalar_tensor_tensor(co[g][:], co[g][:], 1.0, n2g_b[:],
                                               Alu.mult, Alu.mult)
                nc.vector.tensor_tensor(co[g][:], co[g][:], n2b_b[:], Alu.add)

        def transposed_cols(pool, src_list, K, name):
            nk = K // 128
            tT = pool.tile([128, nk * S], f32, name=f"{name}_T")
            for g in range(NG):
                for kc in range(nk):
                    transpose_to(tT[:, kc * S + g * 128: kc * S + (g + 1) * 128],
                                 src_list[g][:, kc * 128:(kc + 1) * 128],
                                 f"{name}T{g}_{kc}")
            return lambda g, kc: tT[:, kc * S + g * 128: kc * S + (g + 1) * 128]

        def big_matmul(pool, lhsT_cols, w_dram, K, N, name, bias_dram=None,
                       const_lhsT=None, out_list=None):
            nk = K // 128
            wsb = pool.tile([128, nk * N], f32, name=f"{name}_wsb")
            for kc in range(nk):
                dma(wsb[:, kc * N:(kc + 1) * N], w_dram[kc * 128:(kc + 1) * 128, :])
            bias_b = (bcast_row(pool, bias_dram, N, f"{name}_bias")
                      if bias_dram is not None else None)
            cvec_b = None
            if const_lhsT is not None:
                cps = pool_ps.tile([1, N], f32, name="cps", tag="Tps",
                                   padded_shape=[128, 512])
                for kc in range(nk):
                    nc.tensor.matmul(cps[:1, :], const_lhsT[:, kc:kc + 1],
                                     wsb[:, kc * N:(kc + 1) * N],
                                     start=(kc == 0), stop=(kc == nk - 1))
                cvec = pool.tile([1, N], f32, name=f"{name}_cvec")
                nc.vector.tensor_copy(cvec[:], cps[:1, :])
                cvec_b = pool.tile([128, N], f32, name=f"{name}_cvecb")
                pbcast(pool, cvec_b[:], cvec[:], N, f"{name}cv")
            outs = []
            for g in range(NG):
                o = (out_list[g] if out_list is not None
                     else pool.tile([128, N], f32, name=f"{name}_o{g}"))
                for nb in range(0, N, 512):
                    nw = min(512, N - nb)
                    ps = pool_mm.tile([128, nw], f32, name="Fps", tag="Fps")
                    for kc in range(nk):
                        nc.tensor.matmul(ps[:], lhsT_cols(g, kc),
                                         wsb[:, kc * N + nb: kc * N + nb + nw],
                                         start=(kc == 0), stop=(kc == nk - 1))
                    nc.vector.tensor_copy(o[:, nb:nb + nw], ps[:])
                if bias_b is not None:
                    nc.vector.tensor_tensor(o[:], o[:], bias_b[:], Alu.add)
                if cvec_b is not None:
                    nc.vector.tensor_tensor(o[:], o[:], cvec_b[:], Alu.add)
                outs.append(o)
            return outs


        # memory-bank mean -> memvT [D,1] as 4 chunks
        with tc.tile_pool(name="tailmem", bufs=1) as mp:
            memx = mp.tile([128, 4 * D], f32, name="memx")
            for kc in range(4):
                dma(memx[:, kc * D:(kc + 1) * D],
                    memory_bank[kc * 128:(kc + 1) * 128, :])
            mem_ps = pool_ps.tile([1, D], f32, name="memps", tag="Tps",
                                  padded_shape=[128, 512])
            for kc in range(4):
                nc.tensor.matmul(mem_ps[:1, :], ones_sb[:],
                                 memx[:, kc * D:(kc + 1) * D],
                                 start=(kc == 0), stop=(kc == 3))
            memv = mp.tile([1, D], f32, name="memv")
            nc.vector.tensor_scalar(memv[:], mem_ps[:1, :], 1.0 / 512.0, None,
                                    Alu.mult)
            memvT = tailP.tile([128, 4], f32, name="memvT")
            for kc in range(4):
                transpose_to(memvT[:, kc:kc + 1], memv[:, kc * 128:(kc + 1) * 128],
                             f"memvT{kc}")

        with tc.tile_pool(name="tailA", bufs=1) as ta_:
            coT = transposed_cols(ta_, co, D, "coT")
            mh = big_matmul(ta_, coT, mem_w1, D, D, "memh", bias_dram=mem_b1,
                            const_lhsT=memvT)
            for g in range(NG):
                silu_(ta_, mh[g][:], mh[g][:], f"mh{g}")
            mhT = transposed_cols(ta_, mh, D, "mhT")
            mo = big_matmul(ta_, mhT, mem_w2, D, D, "memo", bias_dram=mem_b2)
            for g in range(NG):
                nc.vector.tensor_tensor(co[g][:], co[g][:], mo[g][:], Alu.add)

        # FFN: N-sharded across cores (this core's gate|val strips of 256 each)
        HWS = D // 2  # 256-wide gate and val strips
        ffn_stage = [pool_dram.tile([128, D], f32, name=f"ffn_stage{g}")
                     for g in range(NG)]
        ffn_out = [pool_dram.tile([128, D], f32, name=f"ffn_out{g}",
                                  addr_space="Shared") for g in range(NG)]
        gv = [tailP.tile([128, HWS], f32, name=f"gv{g}") for g in range(NG)]
        with tc.tile_pool(name="tailB", bufs=1) as tb_:
            coT2 = transposed_cols(tb_, co, D, "coT2")
            ff = big_matmul(tb_, coT2, up_ws, D, 2 * HWS, "ff", bias_dram=up_bs)
            for g in range(NG):
                silu_(tb_, gv[g][:], ff[g][:, :HWS], f"gv{g}")
                nc.vector.tensor_tensor(gv[g][:], gv[g][:], ff[g][:, HWS:],
                                        Alu.mult)
        with tc.tile_pool(name="tailC", bufs=1) as tcp:
            gvT = tcp.tile([128, 2 * S], f32, name="gvT")
            for g in range(NG):
                for kc in range(2):
                    transpose_to(gvT[:, kc * S + g * 128: kc * S + (g + 1) * 128],
                                 gv[g][:, kc * 128:(kc + 1) * 128], f"gvT{g}_{kc}")
            dwsb = tcp.tile([128, 2 * D], f32, name="dwsb")
            for kc in range(2):
                dma(dwsb[:, kc * D:(kc + 1) * D],
                    down_ws[kc * 128:(kc + 1) * 128, :])
            for g in range(NG):
                ps = pool_mm.tile([128, D], f32, name="Fps", tag="Fps")
                for kc in range(2):
                    nc.tensor.matmul(ps[:], gvT[:, kc * S + g * 128:
                                                 kc * S + (g + 1) * 128],
                                     dwsb[:, kc * D:(kc + 1) * D],
                                     start=(kc == 0), stop=(kc == 1))
                part = tcp.tile([128, D], f32, name=f"part{g}", tag="part")
                nc.vector.tensor_copy(part[:], ps[:])
                dma(ffn_stage[g][:], part[:])
                nc.gpsimd.collective_compute(
                    "AllReduce", Alu.add, replica_groups=RG,
                    ins=[ffn_stage[g][:]], outs=[ffn_out[g][:]])
            dnb_b = bcast_row(tcp, down_b, D, "dnb_b")
            for g in range(NG):
                fsum = tcp.tile([128, D], f32, name=f"fsum{g}", tag="fsum")
                dma(fsum[:], ffn_out[g][:])
                nc.vector.tensor_tensor(fsum[:], fsum[:], dnb_b[:], Alu.add)
                nc.vector.tensor_tensor(fsum[:], fsum[:], co[g][:], Alu.add)
                dma(out_dram[g * 128:(g + 1) * 128, :], fsum[:])

    return nc


def _install_ntff_shim():
    """Reconstitute the missing antenv.axon_hooks module so
    run_bass_kernel_spmd(trace=True) can reach the axon NTFF profiler."""
    import sys
    import types

    if "antenv.axon_hooks" in sys.modules:
        return
    import antenv

    mod = types.ModuleType("antenv.axon_hooks")
    _h = [None]
    mod.set_axon_ntff_profile_hook = lambda h: _h.__setitem__(0, h)
    mod.get_axon_ntff_profile_hook = lambda: _h[0]
    sys.modules["antenv.axon_hooks"] = mod
    antenv.axon_hooks = mod
    try:
        from trn_agent_boot.trn_boot import _ntff_profile_via_ctypes

        mod.set_axon_ntff_profile_hook(
            _ntff_profile_via_ctypes("/opt/axon/libaxon_pjrt.so"))
    except Exception:
        pass


def kernel(**inputs):
    from concourse.bass_utils import run_bass_kernel_spmd
    _install_ntff_shim()

    sin, cos, qpoly = _host_constants()
    x = np.ascontiguousarray(np.asarray(inputs["x"], np.float32).reshape(S, D))
    patterns = np.ascontiguousarray(np.asarray(inputs["flow_patterns"], np.float32))

    nc = build_kernel()
    nc.finalize()

    def a(k):
        return np.ascontiguousarray(np.asarray(inputs[k], np.float32))

    def row(k):
        return np.ascontiguousarray(np.asarray(inputs[k], np.float32).reshape(1, -1))

    up_w = np.asarray(inputs["up_w"], np.float32)      # [D, 8D]
    up_b = np.asarray(inputs["up_b"], np.float32).reshape(-1)
    down_w = np.asarray(inputs["down_w"], np.float32)  # [4D, D]

    base = {
        "x": x,
        "sel_w1": a("sel_w1"), "sel_b1": row("sel_b1"),
        "sel_w2": a("sel_w2"), "sel_b2": row("sel_b2"),
        "win_w1": a("win_w1"), "win_b1": row("win_b1"),
        "win_w2": a("win_w2"), "win_b2": row("win_b2"),
        "int_w1": a("int_w1"), "int_b1": row("int_b1"),
        "int_w2": a("int_w2"), "int_b2": row("int_b2"),
        "mem_w1": a("mem_w1"), "mem_b1": row("mem_b1"),
        "mem_w2": a("mem_w2"), "mem_b2": row("mem_b2"),
        "memory_bank": a("memory_bank"),
        "down_b": row("down_b"),
        "n1_g": row("n1_g"), "n1_b": row("n1_b"),
        "n2_g": row("n2_g"), "n2_b": row("n2_b"),
        "rope_sin": sin, "rope_cos": cos,
        "qpoly": qpoly.reshape(1, 4),
    }
    HWS = D // 2
    in_maps = []
    for c in range(NCORES):
        m = dict(base)
        m["pat_sl"] = np.ascontiguousarray(
            patterns[:, c * ISLICE:(c + 1) * ISLICE, :].reshape(P, FREE))
        gs = slice(c * HWS, (c + 1) * HWS)
        vs = slice(4 * D + c * HWS, 4 * D + (c + 1) * HWS)
        m["up_ws"] = np.ascontiguousarray(
            np.concatenate([up_w[:, gs], up_w[:, vs]], axis=1))
        m["up_bs"] = np.ascontiguousarray(
            np.concatenate([up_b[gs], up_b[vs]]).reshape(1, -1))
        m["down_ws"] = np.ascontiguousarray(down_w[c * HWS:(c + 1) * HWS, :])
        in_maps.append(m)

    trace = os.environ.get("KERNEL_TRACE", "0") == "1"
    res = run_bass_kernel_spmd(nc, in_maps, list(range(NCORES)), trace=trace)
    out0 = res.results[0]
    kernel.last_results = res.results
    kernel.last_exec_ns = getattr(res, "exec_time_ns", None)
    return out0["out"].reshape(B, S, D).astype(np.float32)


if __name__ == "__main__":
    data = np.load("/tmp/inputs.npz")
    inputs = {k: data[k] for k in data.files}
    out = kernel(**inputs)
    print("out", out.shape, float(np.abs(out).max()))


# revision 37
# speedup vs baseline: 1.0275x; 1.0275x over previous
"""Trainium2 Bass kernel for nn_EnhancedFlowLayer (topk_masking).

8 cores. Tokens on partitions (2 groups of 128); flow (i,j)-space sharded by i
across cores (64 i-rows -> 32768 elems/token/core). flow is rematerialized on
the PE per phase (quad-packed: 4 concurrent K=16 matmuls via tile_position)
and never hits HBM. Per-token exact rank-kk threshold via: bf16 |F| + sampled
Newton + exact 5-rung count ladder (counts split across DVE and ACT engines;
ACT counts use Sign at bf16-midpoint boundaries, tie-free and exact) + band
extraction (top-2 per 64-chunk, narrow band) + one all-gather + replicated
exact fp32 bisection (10 rounds, DVE/ACT split) + top-8 peel select. Final
pass recomputes F, applies mask, does the masked matvec, one all-gather of
flow_out slices, then a replicated LN2 + memory-MLP and an N-sharded FFN
(per-core output-column strip + AllReduce of down-proj partials).
"""

import os
from contextlib import ExitStack

import numpy as np

B, S, D, P = 1, 256, 512, 16
MAX_SEQ = 4096
NCORES = 8
ISLICE = D // NCORES          # 64 i-rows per core
FREE = ISLICE * D             # 32768 ij elements per token per core
NG = 2                        # token groups of 128
DD = D * D
TOPL = int(os.environ.get("KERNEL_TOPL", "2"))       # top-L per 64-chunk
BAND = float(os.environ.get("KERNEL_BAND", "450.0"))  # ladder band width (elems)
NCH = FREE // 64              # 512 chunks of 64
NCAND = TOPL * NCH            # candidate slots per token per core
NL = 5                        # ladder rungs
N_BISECT = int(os.environ.get("KERNEL_NBISECT", "8"))
# ladder/bisect engine split (elements handled by DVE vs ACT); all segments
# 8192-wide to stay 16KB-aligned (misaligned reads cost ~20% on DVE counts)
LAD_DVE = 16384               # of 32768 (2 DVE + 2 ACT segments of 8192)
LAD_ACT = (FREE - LAD_DVE) // 2
BIS_DVE = 3584                # bisect DVE count width (rest of 8192 on ACT)

DEBUG = os.environ.get("KERNEL_DEBUG", "0") == "1"
STAGE = int(os.environ.get("KERNEL_STAGE", "3"))
MM_DT_NAME = os.environ.get("KERNEL_MM_DT", "float32")
SIM_COMPAT = os.environ.get("KERNEL_SIM_COMPAT", "0") == "1"


def _host_constants():
    pos = np.arange(S, dtype=np.float64)
    inv = 1.0 / (10000.0 ** (np.arange(0, D, 2, dtype=np.float64) / D))
    ang = pos[:, None] * inv[None, :]
    sin = np.repeat(np.sin(ang), 2, axis=-1).astype(np.float32)
    cos = np.repeat(np.cos(ang), 2, axis=-1).astype(np.float32)
    # half-normal tail quantile z(q): P(|N(0,1)| >= z) = q, cubic in ln q
    qpoly = np.array([-0.0036756, -0.06789169, -0.73664117, 0.26370117], np.float32)
    return sin, cos, qpoly


def build_kernel():
    import concourse.bass as bass
    import concourse.mybir as mybir
    from concourse import bacc, masks
    from concourse.tile import TileContext

    dt = mybir.dt
    Alu = mybir.AluOpType
    Act = mybir.ActivationFunctionType
    AxX = mybir.AxisListType.X
    f32, bf16 = dt.float32, dt.bfloat16
    MM_DT = getattr(dt, MM_DT_NAME)

    nc = bacc.Bacc("TRN2", num_devices=NCORES)

    def mmc(ap):
        return ap.bitcast(MM_DT) if MM_DT != f32 else ap

    dp = nc.declare_dram_parameter
    x_in = dp("x", [S, D], f32, isOutput=False)
    pat_sl = dp("pat_sl", [P, FREE], f32, isOutput=False)
    sel_w1 = dp("sel_w1", [2 * D, 2 * P], f32, isOutput=False)
    sel_b1 = dp("sel_b1", [1, 2 * P], f32, isOutput=False)
    sel_w2 = dp("sel_w2", [2 * P, P], f32, isOutput=False)
    sel_b2 = dp("sel_b2", [1, P], f32, isOutput=False)
    win_w1 = dp("win_w1", [D, 64], f32, isOutput=False)
    win_b1 = dp("win_b1", [1, 64], f32, isOutput=False)
    win_w2 = dp("win_w2", [64, 1], f32, isOutput=False)
    win_b2 = dp("win_b2", [1, 1], f32, isOutput=False)
    int_w1 = dp("int_w1", [2 * D, 64], f32, isOutput=False)
    int_b1 = dp("int_b1", [1, 64], f32, isOutput=False)
    int_w2 = dp("int_w2", [64, 1], f32, isOutput=False)
    int_b2 = dp("int_b2", [1, 1], f32, isOutput=False)
    mem_w1 = dp("mem_w1", [2 * D, D], f32, isOutput=False)
    mem_b1 = dp("mem_b1", [1, D], f32, isOutput=False)
    mem_w2 = dp("mem_w2", [D, D], f32, isOutput=False)
    mem_b2 = dp("mem_b2", [1, D], f32, isOutput=False)
    memory_bank = dp("memory_bank", [512, D], f32, isOutput=False)
    up_ws = dp("up_ws", [D, 2 * (D // 2)], f32, isOutput=False)   # per-core strip
    up_bs = dp("up_bs", [1, 2 * (D // 2)], f32, isOutput=False)
    down_ws = dp("down_ws", [D // 2, D], f32, isOutput=False)     # per-core strip
    down_b = dp("down_b", [1, D], f32, isOutput=False)
    n1_g = dp("n1_g", [1, D], f32, isOutput=False)
    n1_b = dp("n1_b", [1, D], f32, isOutput=False)
    n2_g = dp("n2_g", [1, D], f32, isOutput=False)
    n2_b = dp("n2_b", [1, D], f32, isOutput=False)
    rope_sin = dp("rope_sin", [S, D], f32, isOutput=False)
    rope_cos = dp("rope_cos", [S, D], f32, isOutput=False)
    qpoly = dp("qpoly", [1, 4], f32, isOutput=False)
    out_dram = dp("out", [S, D], f32, isOutput=True)

    dbg = {}
    if DEBUG:
        for name, shape in [
            ("dbg_xn", [S, D]), ("dbg_xr", [S, D]), ("dbg_pw", [S, P]),
            ("dbg_inten", [S, 1]), ("dbg_scal", [1, 8]), ("dbg_t0", [S, 1]),
            ("dbg_cnt", [S, 8]), ("dbg_beta", [S, 4]), ("dbg_th", [S, 2]),
            ("dbg_fo", [S, D]), ("dbg_cand", [S, NCAND]),
        ]:
            dbg[name] = dp(name, shape, f32, isOutput=True)

    RG = [list(range(NCORES))]

    with ExitStack() as ctx:
        tc = ctx.enter_context(TileContext(nc))
        # persistent small state (lives for whole kernel)
        pw_ = ctx.enter_context(tc.tile_pool(name="persist", bufs=1))
        # PSUM pools: 6 banks matmul + 2 banks transposes/misc
        pool_mm = ctx.enter_context(tc.tile_pool(name="psumMM", bufs=6, space="PSUM"))
        pool_ps = ctx.enter_context(tc.tile_pool(name="psumT", bufs=2, space="PSUM"))
        pool_dram = ctx.enter_context(tc.tile_pool(name="dramst", bufs=1, space="DRAM"))

        def dma(dst, src):
            nc.sync.dma_start(out=dst, in_=src)

        def bcast_row(pool, src_dram_row, width, name, dtype=f32):
            t = pool.tile([128, width], dtype, name=name)
            dma(t[:], src_dram_row[:].to_broadcast([128, width]))
            return t

        identity = pw_.tile([128, 128], f32, name="identity")
        masks.make_identity(nc, identity[:])
        bc_n = [0]

        def pbcast(pool, dst_ap, src_ap, width, name):
            """broadcast [1,width] sbuf row to [128,width] via a DRAM bounce"""
            bc_n[0] += 1
            st = pool_dram.tile([1, width], f32, name=f"bc{bc_n[0]}_{name}")
            dma(st[:], src_ap)
            dma(dst_ap, st[:].to_broadcast([128, width]))

        def transpose_to(dst_ap, src_ap, name):
            p, f = src_ap.shape[0], src_ap.free_size()
            ps = pool_ps.tile([f, p], f32, name="Tps", tag="Tps",
                              padded_shape=[128, 128])
            nc.tensor.transpose(ps[:f, :p], src_ap, identity[:p, :p])
            nc.vector.tensor_copy(dst_ap, ps[:f, :p])

        ERF_FN = Act.Tanh if SIM_COMPAT else Act.Erf

        def gelu_(pool, ap, name):
            """in-place exact gelu: x * 0.5*(1+erf(x/sqrt(2)))"""
            e = pool.tile(list(ap.shape), f32, name=f"{name}_erf", tag="gelu_e")
            nc.scalar.activation(e[:], ap, ERF_FN, scale=float(1 / np.sqrt(2)))
            nc.vector.tensor_scalar(e[:], e[:], 1.0, 0.5, Alu.add, Alu.mult)
            nc.vector.tensor_tensor(ap, ap, e[:], Alu.mult)

        def silu_(pool, dst_ap, src_ap, name):
            """dst = src * sigmoid(src) (exact identity)"""
            sg = pool.tile(list(src_ap.shape), f32, name=f"{name}_sg", tag="silu_s")
            nc.scalar.activation(sg[:], src_ap, Act.Sigmoid)
            nc.vector.tensor_tensor(dst_ap, src_ap, sg[:], Alu.mult)

        # ---------- persistent tiles ----------
        xg = [pw_.tile([128, D], f32, name=f"xg{g}") for g in range(NG)]
        xn = [pw_.tile([128, D], f32, name=f"xn{g}") for g in range(NG)]
        pwq = [pw_.tile([128, 128], f32, name=f"pwq{g}") for g in range(NG)]
        inten = [pw_.tile([128, 1], f32, name=f"inten{g}") for g in range(NG)]
        kk_b = pw_.tile([128, 1], f32, name="kk_b")
        zq_b = pw_.tile([128, 1], f32, name="zq_b")
        delta_b = pw_.tile([128, 1], f32, name="delta_b")
        invz2_b = pw_.tile([128, 1], f32, name="invz2_b")
        ones_sb = pw_.tile([128, 1], f32, name="ones_sb")
        nc.vector.memset(ones_sb[:], 1.0)
        beta = [(pw_.tile([128, 1], f32, name=f"b1t{g}"),
                 pw_.tile([128, 1], f32, name=f"b2t{g}")) for g in range(NG)]
        rprime = [pw_.tile([128, 1], f32, name=f"rp{g}") for g in range(NG)]
        th = [pw_.tile([128, 1], f32, name=f"th{g}") for g in range(NG)]

        for g in range(NG):
            dma(xg[g][:], x_in[g * 128:(g + 1) * 128, :])

        # =================== preamble (scoped pool) ===================
        with tc.tile_pool(name="preamble", bufs=1) as pp:
            sin_g, cos_g, xr = [], [], []
            for g in range(NG):
                t = pp.tile([128, D], f32, name=f"sin{g}")
                dma(t[:], rope_sin[g * 128:(g + 1) * 128, :])
                sin_g.append(t)
                t = pp.tile([128, D], f32, name=f"cos{g}")
                dma(t[:], rope_cos[g * 128:(g + 1) * 128, :])
                cos_g.append(t)
            n1g_b = bcast_row(pp, n1_g, D, "n1g_b")
            n1b_b = bcast_row(pp, n1_b, D, "n1b_b")

            for g in range(NG):
                mean = pp.tile([128, 1], f32, name=f"mean{g}")
                m2 = pp.tile([128, 1], f32, name=f"m2ln{g}")
                tmp = pp.tile([128, D], f32, name=f"lntmp{g}")
                nc.vector.tensor_reduce(mean[:], xg[g][:], AxX, Alu.add)
                nc.vector.tensor_scalar(mean[:], mean[:], 1.0 / D, None, Alu.mult)
                nc.vector.tensor_scalar(tmp[:], xg[g][:], mean[:], None, Alu.subtract)
                nc.vector.scalar_tensor_tensor(tmp[:], tmp[:], 1.0, tmp[:], Alu.mult,
                                               Alu.mult, accum_out=m2[:])
                nc.vector.tensor_scalar(m2[:], m2[:], 1.0 / D, 1e-5, Alu.mult, Alu.add)
                rstd = pp.tile([128, 1], f32, name=f"rstd{g}")
                nc.scalar.activation(rstd[:], m2[:], Act.Sqrt)
                nc.vector.reciprocal(rstd[:], rstd[:])
                nc.vector.tensor_scalar(xn[g][:], xg[g][:], mean[:], rstd[:],
                                        Alu.subtract, Alu.mult)
                nc.vector.scalar_tensor_tensor(xn[g][:], xn[g][:], 1.0, n1g_b[:],
                                               Alu.mult, Alu.mult)
                nc.vector.tensor_tensor(xn[g][:], xn[g][:], n1b_b[:], Alu.add)
                t_xr = pp.tile([128, D], f32, name=f"xr{g}")
                rot = pp.tile([128, D], f32, name=f"rot{g}")
                ev = lambda a: a.rearrange("p (a two) -> p a two", two=2)[:, :, 0]
                od = lambda a: a.rearrange("p (a two) -> p a two", two=2)[:, :, 1]
                nc.vector.tensor_scalar(ev(rot[:]), od(xn[g][:]), -1.0, None, Alu.mult)
                nc.vector.tensor_copy(od(rot[:]), ev(xn[g][:]))
                nc.vector.tensor_tensor(rot[:], rot[:], sin_g[g][:], Alu.mult)
                nc.vector.scalar_tensor_tensor(t_xr[:], xn[g][:], 1.0, cos_g[g][:],
                                               Alu.mult, Alu.mult)
                nc.vector.tensor_tensor(t_xr[:], t_xr[:], rot[:], Alu.add)
                xr.append(t_xr)

            # ctx = mean over tokens
            ctx_ps = pool_ps.tile([1, D], f32, name="ctx_ps", tag="Tps",
                                  padded_shape=[128, 512])
            for g in range(NG):
                nc.tensor.matmul(ctx_ps[:1, :], ones_sb[:], xr[g][:],
                                 start=(g == 0), stop=(g == NG - 1))
            ctx_row = pp.tile([1, D], f32, name="ctx_row")
            nc.vector.tensor_scalar(ctx_row[:], ctx_ps[:1, :], 1.0 / S, None, Alu.mult)

            xrT = pp.tile([128, 4 * S], f32, name="xrT")
            for g in range(NG):
                for kc in range(4):
                    transpose_to(xrT[:, kc * S + g * 128: kc * S + (g + 1) * 128],
                                 xr[g][:, kc * 128:(kc + 1) * 128], f"xrT{g}{kc}")
            ctxT = pw_.tile([128, 4], f32, name="ctxT")
            for kc in range(4):
                transpose_to(ctxT[:, kc:kc + 1], ctx_row[:, kc * 128:(kc + 1) * 128],
                             f"ctxT{kc}")

            def mlp_head(w1, b1, w2, b2, h1_dim, h2_dim, name):
                w1a = pp.tile([128, 4 * h1_dim], f32, name=f"{name}_w1a")
                w1b = pp.tile([128, 4 * h1_dim], f32, name=f"{name}_w1b")
                for kc in range(4):
                    dma(w1a[:, kc * h1_dim:(kc + 1) * h1_dim],
                        w1[kc * 128:(kc + 1) * 128, :])
                    dma(w1b[:, kc * h1_dim:(kc + 1) * h1_dim],
                        w1[D + kc * 128: D + (kc + 1) * 128, :])
                b1_b = bcast_row(pp, b1, h1_dim, f"{name}_b1b")
                w2_sb = pp.tile([h1_dim, h2_dim], f32, name=f"{name}_w2sb")
                dma(w2_sb[:], w2[:])
                b2_b = bcast_row(pp, b2, h2_dim, f"{name}_b2b")
                v1_ps = pool_ps.tile([1, h1_dim], f32, name="v1ps", tag="Tps",
                                     padded_shape=[128, 128])
                for kc in range(4):
                    nc.tensor.matmul(v1_ps[:1, :], ctxT[:, kc:kc + 1],
                                     w1b[:, kc * h1_dim:(kc + 1) * h1_dim],
                                     start=(kc == 0), stop=(kc == 3))
                v1 = pp.tile([1, h1_dim], f32, name=f"{name}_v1")
                nc.vector.tensor_copy(v1[:], v1_ps[:1, :])
                v1_b = pp.tile([128, h1_dim], f32, name=f"{name}_v1b")
                pbcast(pp, v1_b[:], v1[:], h1_dim, f"{name}v1")
                outs = []
                for g in range(NG):
                    h1_ps = pool_ps.tile([128, h1_dim], f32, name="h1ps", tag="Tps",
                                         padded_shape=[128, 128])
                    for kc in range(4):
                        nc.tensor.matmul(
                            h1_ps[:], xrT[:, kc * S + g * 128: kc * S + (g + 1) * 128],
                            w1a[:, kc * h1_dim:(kc + 1) * h1_dim],
                            start=(kc == 0), stop=(kc == 3))
                    h1 = pp.tile([128, h1_dim], f32, name=f"{name}_h1_{g}")
                    nc.vector.tensor_tensor(h1[:], h1_ps[:], v1_b[:], Alu.add)
                    nc.vector.tensor_tensor(h1[:], h1[:], b1_b[:], Alu.add)
                    gelu_(pp, h1[:], f"{name}g{g}")
                    h1T = pp.tile([h1_dim, 128], f32, name=f"{name}_h1T_{g}")
                    transpose_to(h1T[:], h1[:], f"{name}h1T{g}")
                    h2_ps = pool_ps.tile([128, h2_dim], f32, name="h2ps", tag="Tps",
                                         padded_shape=[128, 128])
                    nc.tensor.matmul(h2_ps[:], h1T[:], w2_sb[:], start=True, stop=True)
                    h2 = pp.tile([128, h2_dim], f32, name=f"{name}_h2_{g}")
                    nc.vector.tensor_tensor(h2[:], h2_ps[:], b2_b[:], Alu.add)
                    outs.append(h2)
                return outs

            sel_h2 = mlp_head(sel_w1, sel_b1, sel_w2, sel_b2, 2 * P, P, "sel")
            int_h2 = mlp_head(int_w1, int_b1, int_w2, int_b2, 64, 1, "intm")

            for g in range(NG):
                t_pw = pp.tile([128, P], f32, name=f"pwsm{g}")
                mx = pp.tile([128, 1], f32, name=f"selmx{g}")
                nc.vector.tensor_reduce(mx[:], sel_h2[g][:], AxX, Alu.max)
                nc.vector.tensor_scalar(sel_h2[g][:], sel_h2[g][:], mx[:], None,
                                        Alu.subtract)
                nc.scalar.activation(sel_h2[g][:], sel_h2[g][:], Act.Exp)
                sm = pp.tile([128, 1], f32, name=f"selsm{g}")
                nc.vector.tensor_reduce(sm[:], sel_h2[g][:], AxX, Alu.add)
                rs = pp.tile([128, 1], f32, name=f"selrs{g}")
                nc.vector.reciprocal(rs[:], sm[:])
                nc.vector.tensor_scalar(t_pw[:], sel_h2[g][:], rs[:], None, Alu.mult)
                nc.scalar.activation(inten[g][:], int_h2[g][:], Act.Sigmoid)
                # pw^T replicated into the 4 PE row-quadrants (via SBUF->SBUF DMA)
                pwt0 = pp.tile([P, 128], f32, name=f"pwt0_{g}")
                transpose_to(pwt0[:], t_pw[:], f"pwT{g}")
                for q in range(4):
                    dma(pwq[g][32 * q:32 * q + 16, :], pwt0[:])
                if DEBUG:
                    dma(dbg["dbg_pw"][g * 128:(g + 1) * 128, :], t_pw[:])


            # window scalar -> kk, z, delta
            winw1_sb = pp.tile([128, 4 * 64], f32, name="winw1_sb")
            for kc in range(4):
                dma(winw1_sb[:, kc * 64:(kc + 1) * 64],
                    win_w1[kc * 128:(kc + 1) * 128, :])
            wh1_ps = pool_ps.tile([1, 64], f32, name="wh1ps", tag="Tps",
                                  padded_shape=[128, 128])
            for kc in range(4):
                nc.tensor.matmul(wh1_ps[:1, :], ctxT[:, kc:kc + 1],
                                 winw1_sb[:, kc * 64:(kc + 1) * 64],
                                 start=(kc == 0), stop=(kc == 3))
            wh1 = pp.tile([1, 64], f32, name="wh1")
            wb1_sb = pp.tile([1, 64], f32, name="wb1_sb")
            dma(wb1_sb[:], win_b1[:])
            nc.vector.tensor_tensor(wh1[:], wh1_ps[:1, :], wb1_sb[:], Alu.add)
            gelu_(pp, wh1[:], "wh1g")
            wh1T = pp.tile([64, 1], f32, name="wh1T")
            transpose_to(wh1T[:], wh1[:], "wh1T")
            winw2_sb = pp.tile([64, 1], f32, name="winw2_sb")
            dma(winw2_sb[:], win_w2[:])
            win_ps = pool_ps.tile([1, 1], f32, name="winps", tag="Tps",
                                  padded_shape=[128, 128])
            nc.tensor.matmul(win_ps[:1, :1], wh1T[:], winw2_sb[:], start=True,
                             stop=True)
            winv = pp.tile([1, 1], f32, name="winv")
            wb2_sb = pp.tile([1, 1], f32, name="wb2_sb")
            dma(wb2_sb[:], win_b2[:])
            nc.vector.tensor_tensor(winv[:], win_ps[:1, :1], wb2_sb[:], Alu.add)
            nc.scalar.activation(winv[:], winv[:], Act.Sigmoid)
            nc.vector.tensor_scalar(winv[:], winv[:], float(MAX_SEQ - 256), 256.0,
                                    Alu.mult, Alu.add)
            kkf = pp.tile([1, 1], f32, name="kkf")
            nc.vector.tensor_scalar(kkf[:], winv[:], 0.1 / MAX_SEQ * DD, None,
                                    Alu.mult)
            # floor() robust to the f32->i32 convert rounding mode
            ki = pp.tile([1, 1], dt.int32, name="ki")
            nc.vector.tensor_copy(ki[:], kkf[:])
            kf2 = pp.tile([1, 1], f32, name="kf2")
            nc.vector.tensor_copy(kf2[:], ki[:])
            kgt = pp.tile([1, 1], f32, name="kgt")
            nc.vector.tensor_tensor(kgt[:], kf2[:], kkf[:], Alu.is_gt)
            nc.vector.tensor_tensor(kkf[:], kf2[:], kgt[:], Alu.subtract)
            nc.vector.tensor_scalar(kkf[:], kkf[:], 1.0, None, Alu.max)

            qp = pp.tile([1, 4], f32, name="qp")
            dma(qp[:], qpoly[:])
            u = pp.tile([1, 1], f32, name="qu")
            nc.vector.tensor_scalar(u[:], kkf[:], 1.0 / DD, None, Alu.mult)
            nc.scalar.activation(u[:], u[:], Act.Ln)
            zq = pp.tile([1, 1], f32, name="zq")
            nc.vector.tensor_scalar(zq[:], qp[:, 0:1], u[:], qp[:, 1:2], Alu.mult,
                                    Alu.add)
            nc.vector.tensor_scalar(zq[:], zq[:], u[:], qp[:, 2:3], Alu.mult, Alu.add)
            nc.vector.tensor_scalar(zq[:], zq[:], u[:], qp[:, 3:4], Alu.mult, Alu.add)
            phi = pp.tile([1, 1], f32, name="phi")
            nc.vector.scalar_tensor_tensor(phi[:], zq[:], -0.5, zq[:], Alu.mult,
                                           Alu.mult)
            nc.scalar.activation(phi[:], phi[:], Act.Exp)
            nc.vector.tensor_scalar(phi[:], phi[:], float(1.0 / np.sqrt(2 * np.pi)),
                                    None, Alu.mult)
            dens = pp.tile([1, 1], f32, name="dens")
            nc.vector.scalar_tensor_tensor(dens[:], phi[:], float(2.0 * DD), zq[:],
                                           Alu.mult, Alu.mult)
            delta = pp.tile([1, 1], f32, name="delta")
            nc.vector.reciprocal(delta[:], dens[:])
            nc.vector.tensor_scalar(delta[:], delta[:], BAND, None, Alu.mult)
            pbcast(pp, kk_b[:], kkf[:], 1, "kk")
            pbcast(pp, zq_b[:], zq[:], 1, "zq")
            pbcast(pp, delta_b[:], delta[:], 1, "delta")
            nc.vector.scalar_tensor_tensor(invz2_b[:], zq_b[:], 1.0, zq_b[:],
                                           Alu.mult, Alu.mult)
            nc.vector.reciprocal(invz2_b[:], invz2_b[:])
            if DEBUG:
                dma(dbg["dbg_scal"][:, 0:1], kkf[:])
                dma(dbg["dbg_scal"][:, 1:2], winv[:])
                dma(dbg["dbg_scal"][:, 2:3], zq[:])
                dma(dbg["dbg_scal"][:, 3:4], delta[:])

            if DEBUG:
                for g in range(NG):
                    dma(dbg["dbg_xn"][g * 128:(g + 1) * 128, :], xn[g][:])
                    dma(dbg["dbg_xr"][g * 128:(g + 1) * 128, :], xr[g][:])
                    dma(dbg["dbg_inten"][g * 128:(g + 1) * 128, :], inten[g][:])

        if STAGE < 2:
            for g in range(NG):
                dma(out_dram[g * 128:(g + 1) * 128, :], xg[g][:])
            return nc

        # ====== helper: quad-packed F rematerialization (stream patterns) ======
        def flow_quad(g, consume, pat_pool, swlist=None):
            """consume(c, psum_ap) for each 512-chunk c (i_loc = c) of group g.

            4 concurrent K=16 matmuls on the PE row-quadrants; super-window sw
            covers chunks [16*sw, 16*sw+16).
            """
            for sw in (swlist if swlist is not None else range(4)):
                patq = pat_pool.tile([128, 2048], f32, name="patq", tag="patq",
                                     bufs=3)
                for q in range(4):
                    w = sw * 4 + q
                    dma(patq[32 * q:32 * q + 16, :],
                        pat_sl[:, w * 2048:(w + 1) * 2048])
                for m in range(4):
                    for q in range(4):
                        c = (sw * 4 + q) * 4 + m
                        ps = pool_mm.tile([128, 512], f32, name="Fps", tag="Fps")
                        nc.tensor.matmul(
                            ps[:], mmc(pwq[g][32 * q:32 * q + 16, :]),
                            mmc(patq[32 * q:32 * q + 16, m * 512:(m + 1) * 512]),
                            start=True, stop=True, tile_position=(32 * q, 0))
                        consume(c, ps)

        t0_stage = [pool_dram.tile([128, 1], f32, name=f"t0_stage{g}")
                    for g in range(NG)]
        t0_out = [pool_dram.tile([128, 1], f32, name=f"t0_out{g}",
                                 addr_space="Shared") for g in range(NG)]
        cnt_stage = [pool_dram.tile([128, NL], f32, name=f"cnt_stage{g}")
                     for g in range(NG)]
        cnt_out = [pool_dram.tile([128, NL], f32, name=f"cnt_out{g}",
                                  addr_space="Shared") for g in range(NG)]
        cand_stage = [pool_dram.tile([128, NCAND], f32, name=f"cand_stage{g}")
                      for g in range(NG)]
        cand_out = [pool_dram.tile([NCORES, 128, NCAND], f32, name=f"cand_out{g}",
                                   addr_space="Shared") for g in range(NG)]

        tlad_all = []
        # =============== P1 + selection ladder (scoped pool) ===============
        with tc.tile_pool(name="selpool", bufs=1) as sp:
            A_bf = sp.tile([128, NG * FREE], bf16, name="A_bf")
            scr_d = sp.tile([128, LAD_DVE // 2], bf16, name="scr_d")
            scr_a = sp.tile([128, LAD_ACT], bf16, name="scr_a")

            for g in range(NG):
                def consume_p1(c, ps, g=g):
                    dst = A_bf[:, g * FREE + c * 512: g * FREE + (c + 1) * 512]
                    nc.scalar.activation(dst, ps[:], Act.Abs,
                                         scale=inten[g][:])
                flow_quad(g, consume_p1, sp)


            # moments + Newton per group (both on DVE; groups independent)
            for g in range(NG):
                Ag = A_bf[:, g * FREE:(g + 1) * FREE]
                m4 = sp.tile([128, 4], f32, name=f"m4_{g}")
                for q in range(4):
                    nc.vector.scalar_tensor_tensor(
                        scr_d[:, :8192], Ag[:, q * 8192:(q + 1) * 8192], 1.0,
                        Ag[:, q * 8192:(q + 1) * 8192], Alu.mult, Alu.mult,
                        accum_out=m4[:, q:q + 1])
                m2a = sp.tile([128, 1], f32, name=f"m2a{g}")
                nc.vector.tensor_reduce(m2a[:], m4[:], AxX, Alu.add)
                sig = sp.tile([128, 1], f32, name=f"sig{g}")
                nc.vector.tensor_scalar(sig[:], m2a[:], 1.0 / FREE, None, Alu.mult)
                nc.scalar.activation(sig[:], sig[:], Act.Sqrt)
                t0 = sp.tile([128, 1], f32, name=f"t0{g}")
                nc.vector.tensor_tensor(t0[:], sig[:], zq_b[:], Alu.mult)

                # Newton on a 1/4 chunk-contiguous sample (8192 elems)
                Asmp = Ag.rearrange("p (a b c) -> p a b c", b=4, c=512)[:, :, 0, :]
                cs = sp.tile([128, 1], f32, name=f"cs{g}")
                lnr = sp.tile([128, 1], f32, name=f"lnr{g}")
                ktgt = sp.tile([128, 1], f32, name=f"ktgt{g}")
                nc.vector.tensor_scalar(ktgt[:], kk_b[:], 1.0 / 32.0, None, Alu.mult)
                rtg = sp.tile([128, 1], f32, name=f"rtg{g}")
                nc.vector.reciprocal(rtg[:], ktgt[:])
                scr_s = scr_d[:, :8192].rearrange("p (a c) -> p a c", c=512)
                for it in range(4):
                    nc.vector.tensor_scalar(scr_s, Asmp, t0[:],
                                            None, Alu.is_ge, Alu.add, accum_out=cs[:])
                    nc.vector.tensor_scalar(cs[:], cs[:], 1.0, None, Alu.max)
                    nc.vector.tensor_tensor(lnr[:], cs[:], rtg[:], Alu.mult)
                    nc.vector.tensor_scalar(lnr[:], lnr[:], 0.1, 10.0, Alu.max,
                                            Alu.min)
                    nc.scalar.activation(lnr[:], lnr[:], Act.Ln)
                    nc.vector.tensor_tensor(lnr[:], lnr[:], invz2_b[:], Alu.mult)
                    nc.scalar.activation(lnr[:], lnr[:], Act.Exp)
                    nc.vector.tensor_tensor(t0[:], t0[:], lnr[:], Alu.mult)
                dma(t0_stage[g][:], t0[:])
                # harmonize t0 across cores per group (overlaps next group's
                # moments/Newton; ladders must be identical everywhere)
                nc.gpsimd.collective_compute(
                    "AllReduce", Alu.add, replica_groups=RG,
                    ins=[t0_stage[g][:]], outs=[t0_out[g][:]])

            for g in range(NG):
                Ag = A_bf[:, g * FREE:(g + 1) * FREE]
                t0 = sp.tile([128, 1], f32, name=f"t0h{g}")
                dma(t0[:], t0_out[g][:])
                nc.vector.tensor_scalar(t0[:], t0[:], 1.0 / NCORES, None, Alu.mult)
                if DEBUG:
                    dma(dbg["dbg_t0"][g * 128:(g + 1) * 128, :], t0[:])

                tl = pw_.tile([128, NL], f32, name=f"tlad{g}")
                tl_bf = sp.tile([128, NL], bf16, name=f"tladbf{g}")
                fac = sp.tile([128, 1], f32, name=f"fac{g}")
                for j in range(NL):
                    nc.vector.tensor_scalar(fac[:], delta_b[:], float(j - NL // 2),
                                            None, Alu.mult)
                    nc.scalar.activation(fac[:], fac[:], Act.Exp)
                    nc.vector.tensor_tensor(tl[:, j:j + 1], t0[:], fac[:], Alu.mult)
                nc.vector.tensor_copy(tl_bf[:], tl[:])
                nc.vector.tensor_copy(tl[:], tl_bf[:])
                tlad_all.append(tl)

                # beta-midpoint thresholds for the ACT Sign counts (tie-free)
                nbeta = sp.tile([128, NL], f32, name=f"nbeta{g}")
                pvl = sp.tile([128, NL], f32, name=f"pvl{g}")
                pvl_bf = sp.tile([128, NL], bf16, name=f"pvlbf{g}")
                nc.vector.tensor_scalar(pvl[:], tl[:], float(1.0 - 2.0 ** -8), None,
                                        Alu.mult)
                nc.vector.tensor_copy(pvl_bf[:], pvl[:])
                nc.vector.tensor_copy(pvl[:], pvl_bf[:])
                nc.vector.tensor_tensor(pvl[:], pvl[:], tl[:], Alu.add)
                nc.vector.tensor_scalar(nbeta[:], pvl[:], -0.5, None, Alu.mult)

                cl = sp.tile([128, NL], f32, name=f"cl{g}")
                HD = LAD_DVE // 2
                for j in range(NL):
                    # per-rung accumulator tile so rungs pipeline (no WAR chain)
                    acc4 = sp.tile([128, 4], f32, name=f"acc4_{g}_{j}",
                                   tag="acc4", bufs=4)
                    # DVE: 2 segments (is_ge, ties counted)
                    for s2 in range(2):
                        nc.vector.tensor_scalar(
                            scr_d[:, :HD], Ag[:, s2 * HD:(s2 + 1) * HD],
                            tl[:, j:j + 1], None, Alu.is_ge, Alu.add,
                            accum_out=acc4[:, s2:s2 + 1])
                    # ACT: 2 segments (Sign at beta midpoint, tie-free)
                    for s2 in range(2):
                        lo = LAD_DVE + s2 * LAD_ACT
                        nc.scalar.activation(
                            scr_a[:, :LAD_ACT], Ag[:, lo:lo + LAD_ACT], Act.Sign,
                            bias=nbeta[:, j:j + 1],
                            accum_out=acc4[:, 2 + s2:3 + s2])
                    nc.vector.tensor_tensor(cl[:, j:j + 1], acc4[:, 0:1],
                                            acc4[:, 1:2], Alu.add)
                    csum = sp.tile([128, 1], f32, name=f"csum{g}_{j}", tag="csum",
                                   bufs=4)
                    nc.vector.tensor_tensor(csum[:], acc4[:, 2:3], acc4[:, 3:4],
                                            Alu.add)
                    nc.vector.tensor_scalar(csum[:], csum[:],
                                            float(2 * LAD_ACT), 0.5, Alu.add,
                                            Alu.mult)
                    nc.vector.tensor_tensor(cl[:, j:j + 1], cl[:, j:j + 1], csum[:],
                                            Alu.add)
                dma(cnt_stage[g][:], cl[:])
                # per-group count AllReduce: g0's reduce hides under g1's ladder
                nc.gpsimd.collective_compute(
                    "AllReduce", Alu.add, replica_groups=RG,
                    ins=[cnt_stage[g][:]], outs=[cnt_out[g][:]])

        tailP = ctx.enter_context(tc.tile_pool(name="tailP", bufs=1))
        # ====== P3 pool opens early: quarter (g0,sw0) remat overlaps bracket ======
        p3ctx = tc.tile_pool(name="p3pool", bufs=1)
        xp = p3ctx.__enter__()
        Xq00 = xp.tile([128, 8192], f32, name="Xq", tag="Xq", bufs=2)

        def consume_pre(c, ps):
            nc.scalar.activation(Xq00[:, c * 512:(c + 1) * 512],
                                 ps[:], Act.Abs, scale=inten[0][:])
        flow_quad(0, consume_pre, xp, swlist=[0])

        # bracket selection (small persistent tiles)
        with tc.tile_pool(name="bracket", bufs=1) as bp:
            for g in range(NG):
                cl = bp.tile([128, NL], f32, name=f"clg{g}")
                dma(cl[:], cnt_out[g][:])
                if DEBUG:
                    dma(dbg["dbg_cnt"][g * 128:(g + 1) * 128, 0:NL], cl[:])
                ge = bp.tile([128, NL], f32, name=f"ge{g}")
                nc.vector.tensor_scalar(ge[:], cl[:], kk_b[:], None, Alu.is_ge)
                sel = bp.tile([128, NL - 1], f32, name=f"sel{g}")
                nc.vector.tensor_scalar(sel[:], ge[:, 1:NL], -1.0, 1.0, Alu.mult,
                                        Alu.add)
                nc.vector.tensor_tensor(sel[:], sel[:], ge[:, 0:NL - 1], Alu.mult)
                t1 = bp.tile([128, 1], f32, name=f"t1_{g}")
                t2 = bp.tile([128, 1], f32, name=f"t2_{g}")
                c2 = bp.tile([128, 1], f32, name=f"c2_{g}")
                stmp = bp.tile([128, NL - 1], f32, name=f"stmp{g}")
                tl = tlad_all[g]
                nc.vector.tensor_tensor(stmp[:], sel[:], tl[:, 0:NL - 1], Alu.mult)
                nc.vector.tensor_reduce(t1[:], stmp[:], AxX, Alu.add)
                nc.vector.tensor_tensor(stmp[:], sel[:], tl[:, 1:NL], Alu.mult)
                nc.vector.tensor_reduce(t2[:], stmp[:], AxX, Alu.add)
                nc.vector.tensor_tensor(stmp[:], sel[:], cl[:, 1:NL], Alu.mult)
                nc.vector.tensor_reduce(c2[:], stmp[:], AxX, Alu.add)
                # exact fp32 count-boundary of a bf16 threshold t:
                # beta = (t + prev16(t))/2 with prev16(t) = bf16RTN(t*(1-2^-8))
                pv = bp.tile([128, 2], f32, name=f"pv{g}")
                pv_bf = bp.tile([128, 2], bf16, name=f"pvbf{g}")
                nc.vector.tensor_scalar(pv[:, 0:1], t1[:],
                                        float(1.0 - 2.0 ** -8), None, Alu.mult)
                nc.vector.tensor_scalar(pv[:, 1:2], t2[:],
                                        float(1.0 - 2.0 ** -8), None, Alu.mult)
                nc.vector.tensor_copy(pv_bf[:], pv[:])
                nc.vector.tensor_copy(pv[:], pv_bf[:])
                nc.vector.tensor_tensor(pv[:, 0:1], pv[:, 0:1], t1[:], Alu.add)
                nc.vector.tensor_tensor(pv[:, 1:2], pv[:, 1:2], t2[:], Alu.add)
                nc.vector.tensor_scalar(beta[g][0][:], pv[:, 0:1], 0.5, None,
                                        Alu.mult)
                nc.vector.tensor_scalar(beta[g][1][:], pv[:, 1:2], 0.5, None,
                                        Alu.mult)
                nc.vector.scalar_tensor_tensor(rprime[g][:], c2[:], -1.0, kk_b[:],
                                               Alu.mult, Alu.add)
                if DEBUG:
                    dma(dbg["dbg_beta"][g * 128:(g + 1) * 128, 0:1], beta[g][0][:])
                    dma(dbg["dbg_beta"][g * 128:(g + 1) * 128, 1:2], beta[g][1][:])
                    dma(dbg["dbg_beta"][g * 128:(g + 1) * 128, 2:3], c2[:])
                    dma(dbg["dbg_beta"][g * 128:(g + 1) * 128, 3:4], rprime[g][:])

        # ====== P3: band extraction (top-2 per 64-chunk, quarter pipeline) ======
        if True:
            for g in range(NG):
                b1t, b2t = beta[g]
                cand = xp.tile([128, NCAND], f32, name="cand", tag="cand")
                for sw in range(4):
                    if g == 0 and sw == 0:
                        Xq = Xq00
                    else:
                        Xq = xp.tile([128, 8192], f32, name="Xq", tag="Xq",
                                     bufs=2)
                    Zq = xp.tile([128, 8192], f32, name="Zq", tag="Zq", bufs=2)

                    def consume_p3(c, ps, g=g, Xq=Xq, sw=sw):
                        cc = c - sw * 16
                        nc.scalar.activation(Xq[:, cc * 512:(cc + 1) * 512],
                                             ps[:], Act.Abs, scale=inten[g][:])
                    if not (g == 0 and sw == 0):
                        flow_quad(g, consume_p3, xp, swlist=[sw])
                    nc.vector.scalar_tensor_tensor(Zq[:], Xq[:], b2t[:], Xq[:],
                                                   Alu.is_lt, Alu.mult)
                    ch = lambda a: a.rearrange("p (c e) -> p c e", e=64)
                    NQ = 128  # 64-chunks per quarter
                    L1 = xp.tile([128, NQ], f32, name="L1", tag="L1", bufs=2)
                    nc.vector.tensor_reduce(L1[:], ch(Zq[:]), AxX, Alu.max)
                    L1b = L1[:].rearrange("p (c one) -> p c one", one=1).to_broadcast(
                        [128, NQ, 64])
                    nc.vector.tensor_tensor(ch(Xq[:]), ch(Zq[:]), L1b, Alu.is_lt)
                    nc.vector.tensor_tensor(Zq[:], Zq[:], Xq[:], Alu.mult)
                    L2 = xp.tile([128, NQ], f32, name="L2", tag="L2", bufs=2)
                    nc.vector.tensor_reduce(L2[:], ch(Zq[:]), AxX, Alu.max)
                    nc.vector.scalar_tensor_tensor(L1[:], L1[:], b1t[:], L1[:],
                                                   Alu.is_ge, Alu.mult)
                    nc.vector.scalar_tensor_tensor(L2[:], L2[:], b1t[:], L2[:],
                                                   Alu.is_ge, Alu.mult)
                    nc.vector.tensor_copy(cand[:, sw * NQ:(sw + 1) * NQ], L1[:])
                    nc.vector.tensor_copy(cand[:, 512 + sw * NQ:512 + (sw + 1) * NQ],
                                          L2[:])
                dma(cand_stage[g][:], cand[:])
                nc.gpsimd.collective_compute(
                    "AllGather", Alu.bypass, replica_groups=RG,
                    ins=[cand_stage[g][:]], outs=[cand_out[g][:]])
            p3ctx.__exit__(None, None, None)

        # ========= exact threshold: replicated bisection (DVE+ACT split), =========
        # ========= interleaved with P4 so P4-g0 weaves into bisect-g1     =========
        GW = NCORES * NCAND
        fo_stage = pool_dram.tile([S, ISLICE], f32, name="fo_stage")
        fo_out = pool_dram.tile([NCORES, S, ISLICE], f32, name="fo_out",
                                addr_space="Shared")
        fo_full = [tailP.tile([128, D], f32, name=f"fo_full{g}") for g in range(NG)]
        with tc.tile_pool(name="bisect", bufs=1) as gp, \
                tc.tile_pool(name="p4pool", bufs=1) as fp:
            XI = []
            for g in range(NG):
                t = fp.tile([128, D], f32, name=f"XI{g}")
                nc.vector.tensor_scalar(t[:], xn[g][:], inten[g][:], None, Alu.mult)
                XI.append(t)
            T = {}
            for g in range(NG):
                T[g] = dict(
                    G=gp.tile([128, GW], f32, name=f"Gc{g}"),
                    gsc=gp.tile([128, GW], f32, name=f"gsc{g}"),
                    lo=gp.tile([128, 1], f32, name=f"lo{g}"),
                    hi=gp.tile([128, 1], f32, name=f"hi{g}"),
                    mid=gp.tile([128, 1], f32, name=f"mid{g}"),
                    nmid=gp.tile([128, 1], f32, name=f"nmid{g}"),
                    cm=gp.tile([128, 1], f32, name=f"cm{g}"),
                    cma=gp.tile([128, 1], f32, name=f"cma{g}"),
                    sl=gp.tile([128, 1], f32, name=f"sl{g}"),
                    dm=gp.tile([128, 1], f32, name=f"dm{g}"),
                    dh=gp.tile([128, 1], f32, name=f"dh{g}"),
                )
                t = T[g]
                for cidx in range(NCORES):
                    dma(t["G"][:, cidx * NCAND:(cidx + 1) * NCAND],
                        cand_out[g][cidx, :, :])
                if DEBUG and g == 0:
                    dma(dbg["dbg_cand"][0:128, :], t["G"][:, 0:NCAND])
                nc.vector.tensor_copy(t["lo"][:], beta[g][0][:])
                nc.vector.tensor_copy(t["hi"][:], beta[g][1][:])

                G, gsc = t["G"], t["gsc"]
                lo, hi, mid, nmid = t["lo"], t["hi"], t["mid"], t["nmid"]
                cm, cma, sl, dm, dh = (t["cm"], t["cma"], t["sl"], t["dm"],
                                       t["dh"])
                for _ in range(N_BISECT):
                    nc.vector.tensor_tensor(mid[:], lo[:], hi[:], Alu.add)
                    nc.vector.tensor_scalar(mid[:], mid[:], 0.5, None, Alu.mult)
                    nc.vector.tensor_scalar(nmid[:], mid[:], -1.0, None, Alu.mult)
                    # DVE slice + ACT slice, concurrently
                    nc.vector.tensor_scalar(gsc[:, :BIS_DVE], G[:, :BIS_DVE],
                                            mid[:], None, Alu.is_ge, Alu.add,
                                            accum_out=cm[:])
                    nc.scalar.activation(gsc[:, BIS_DVE:], G[:, BIS_DVE:],
                                         Act.Sign, bias=nmid[:],
                                         accum_out=cma[:])
                    nc.vector.tensor_scalar(cma[:], cma[:], float(GW - BIS_DVE),
                                            0.5, Alu.add, Alu.mult)
                    nc.vector.tensor_tensor(cm[:], cm[:], cma[:], Alu.add)
                    # fused interval update: sl = (cm>=r'); lo += (mid-lo)*sl;
                    # hi = mid + (hi-mid)*sl
                    nc.vector.tensor_scalar(sl[:], cm[:], rprime[g][:], None,
                                            Alu.is_ge)
                    nc.vector.tensor_tensor(dm[:], mid[:], lo[:], Alu.subtract)
                    nc.vector.scalar_tensor_tensor(lo[:], dm[:], sl[:], lo[:],
                                                   Alu.mult, Alu.add)
                    nc.vector.tensor_tensor(dh[:], hi[:], mid[:], Alu.subtract)
                    nc.vector.scalar_tensor_tensor(hi[:], dh[:], sl[:], mid[:],
                                                   Alu.mult, Alu.add)

                # cHI = count(G >= hi) (exact, DVE)
                cHI = gp.tile([128, 1], f32, name=f"cHI{g}")
                nc.vector.tensor_scalar(gsc[:], G[:], hi[:], None, Alu.is_ge,
                                        Alu.add, accum_out=cHI[:])
                # window-mask G below hi only; below-lo values are harmless
                # for the count-based rank select (always smaller than window)
                nc.vector.scalar_tensor_tensor(G[:], G[:], hi[:], G[:], Alu.is_lt,
                                               Alu.mult)
                W8 = gp.tile([128, 8], f32, name=f"W8{g}")
                nc.vector.max(out=W8[:], in_=G[:])
                # idx = rprime - cHI; th = idx-th largest of W8 (duplicate-safe:
                # th = max{v in W8 : count(W8 >= v) >= idx}), fallback hi if
                # idx <= 0
                idx = gp.tile([128, 1], f32, name=f"idx{g}")
                nc.vector.scalar_tensor_tensor(idx[:], cHI[:], -1.0, rprime[g][:],
                                               Alu.mult, Alu.add)
                c8 = gp.tile([128, 8], f32, name=f"c8{g}")
                scr8 = gp.tile([128, 8], f32, name=f"scr8{g}")
                for r in range(8):
                    nc.vector.tensor_scalar(scr8[:], W8[:], W8[:, r:r + 1], None,
                                            Alu.is_ge, Alu.add,
                                            accum_out=c8[:, r:r + 1])
                nc.vector.tensor_scalar(c8[:], c8[:], idx[:], None, Alu.is_ge)
                nc.vector.tensor_tensor(c8[:], c8[:], W8[:], Alu.mult)
                vsel = gp.tile([128, 1], f32, name=f"vsel{g}")
                nc.vector.tensor_reduce(vsel[:], c8[:], AxX, Alu.max)
                acc = gp.tile([128, 1], f32, name=f"thacc{g}")
                msk = gp.tile([128, 1], f32, name=f"thmsk{g}")
                nc.vector.tensor_scalar(msk[:], idx[:], 0.5, None, Alu.is_le)
                nc.vector.tensor_tensor(acc[:], msk[:], hi[:], Alu.mult)
                nc.vector.tensor_scalar(msk[:], msk[:], -1.0, 1.0, Alu.mult,
                                        Alu.add)
                nc.vector.tensor_tensor(msk[:], msk[:], vsel[:], Alu.mult)
                nc.vector.tensor_tensor(acc[:], acc[:], msk[:], Alu.add)
                nc.vector.tensor_copy(th[g][:], acc[:])
                if DEBUG:
                    dma(dbg["dbg_th"][g * 128:(g + 1) * 128, 0:1], th[g][:])
                    dma(dbg["dbg_th"][g * 128:(g + 1) * 128, 1:2], rprime[g][:])

            # ---- P4: final masked matvec (after both groups' thresholds)
            if STAGE >= 3:
                for g in range(NG):
                    FO = fp.tile([128, ISLICE], f32, name=f"FO{g}")

                    def consume_p4(c, ps, g=g, FO=FO):
                        At = fp.tile([128, 512], f32, name="At", tag="At", bufs=6)
                        FM = fp.tile([128, 512], f32, name="FM", tag="FM", bufs=6)
                        nc.scalar.activation(At[:], ps[:], Act.Abs,
                                             scale=inten[g][:])
                        nc.vector.scalar_tensor_tensor(FM[:], At[:], th[g][:],
                                                       ps[:], Alu.is_ge, Alu.mult)
                        nc.vector.scalar_tensor_tensor(FM[:], FM[:], 1.0, XI[g][:],
                                                       Alu.mult, Alu.mult,
                                                       accum_out=FO[:, c:c + 1])
                    flow_quad(g, consume_p4, fp)
                    dma(fo_stage[g * 128:(g + 1) * 128, :], FO[:])

        if STAGE < 3:
            for g in range(NG):
                dma(out_dram[g * 128:(g + 1) * 128, :], xg[g][:])
            return nc

        nc.gpsimd.collective_compute(
            "AllGather", Alu.bypass, replica_groups=RG,
            ins=[fo_stage[:]], outs=[fo_out[:]])

        # =============== tail ===============
        co = [tailP.tile([128, D], f32, name=f"co{g}") for g in range(NG)]
        with tc.tile_pool(name="tail1", bufs=1) as tp:
            n2g_b = bcast_row(tp, n2_g, D, "n2g_b")
            n2b_b = bcast_row(tp, n2_b, D, "n2b_b")
            for g in range(NG):
                for cidx in range(NCORES):
                    dma(fo_full[g][:, cidx * ISLICE:(cidx + 1) * ISLICE],
                        fo_out[cidx, g * 128:(g + 1) * 128, :])
                if DEBUG:
                    dma(dbg["dbg_fo"][g * 128:(g + 1) * 128, :], fo_full[g][:])
                nc.vector.tensor_tensor(co[g][:], xg[g][:], fo_full[g][:], Alu.add)
                mean = tp.tile([128, 1], f32, name=f"mean2{g}")
                m2 = tp.tile([128, 1], f32, name=f"m2ln2{g}")
                tmp = tp.tile([128, D], f32, name=f"ln2tmp{g}", tag="tmp")
                nc.vector.tensor_reduce(mean[:], co[g][:], AxX, Alu.add)
                nc.vector.tensor_scalar(mean[:], mean[:], 1.0 / D, None, Alu.mult)
                nc.vector.tensor_scalar(tmp[:], co[g][:], mean[:], None,
                                        Alu.subtract)
                nc.vector.scalar_tensor_tensor(tmp[:], tmp[:], 1.0, tmp[:], Alu.mult,
                                               Alu.mult, accum_out=m2[:])
                nc.vector.tensor_scalar(m2[:], m2[:], 1.0 / D, 1e-5, Alu.mult,
                                        Alu.add)
                rstd = tp.tile([128, 1], f32, name=f"rstd2{g}")
                nc.scalar.activation(rstd[:], m2[:], Act.Sqrt)
                nc.vector.reciprocal(rstd[:], rstd[:])
                nc.vector.tensor_scalar(co[g][:], co[g][:], mean[:], rstd[:],
                                        Alu.subtract, Alu.mult)
                nc.vector.scalar_tensor_tensor(co[g][:], co[g][:], 1.0, n2g_b[:],
                                               Alu.mult, Alu.mult)
                nc.vector.tensor_tensor(co[g][:], co[g][:], n2b_b[:], Alu.add)

        def transposed_cols(pool, src_list, K, name):
            nk = K // 128
            tT = pool.tile([128, nk * S], f32, name=f"{name}_T")
            for g in range(NG):
                for kc in range(nk):
                    transpose_to(tT[:, kc * S + g * 128: kc * S + (g + 1) * 128],
                                 src_list[g][:, kc * 128:(kc + 1) * 128],
                                 f"{name}T{g}_{kc}")
            return lambda g, kc: tT[:, kc * S + g * 128: kc * S + (g + 1) * 128]

        def big_matmul(pool, lhsT_cols, w_dram, K, N, name, bias_dram=None,
                       const_lhsT=None, out_list=None):
            nk = K // 128
            wsb = pool.tile([128, nk * N], f32, name=f"{name}_wsb")
            for kc in range(nk):
                dma(wsb[:, kc * N:(kc + 1) * N], w_dram[kc * 128:(kc + 1) * 128, :])
            bias_b = (bcast_row(pool, bias_dram, N, f"{name}_bias")
                      if bias_dram is not None else None)
            cvec_b = None
            if const_lhsT is not None:
                cps = pool_ps.tile([1, N], f32, name="cps", tag="Tps",
                                   padded_shape=[128, 512])
                for kc in range(nk):
                    nc.tensor.matmul(cps[:1, :], const_lhsT[:, kc:kc + 1],
                                     wsb[:, kc * N:(kc + 1) * N],
                                     start=(kc == 0), stop=(kc == nk - 1))
                cvec = pool.tile([1, N], f32, name=f"{name}_cvec")
                nc.vector.tensor_copy(cvec[:], cps[:1, :])
                cvec_b = pool.tile([128, N], f32, name=f"{name}_cvecb")
                pbcast(pool, cvec_b[:], cvec[:], N, f"{name}cv")
            outs = []
            for g in range(NG):
                o = (out_list[g] if out_list is not None
                     else pool.tile([128, N], f32, name=f"{name}_o{g}"))
                for nb in range(0, N, 512):
                    nw = min(512, N - nb)
                    ps = pool_mm.tile([128, nw], f32, name="Fps", tag="Fps")
                    for kc in range(nk):
                        nc.tensor.matmul(ps[:], lhsT_cols(g, kc),
                                         wsb[:, kc * N + nb: kc * N + nb + nw],
                                         start=(kc == 0), stop=(kc == nk - 1))
                    nc.vector.tensor_copy(o[:, nb:nb + nw], ps[:])
                if bias_b is not None:
                    nc.vector.tensor_tensor(o[:], o[:], bias_b[:], Alu.add)
                if cvec_b is not None:
                    nc.vector.tensor_tensor(o[:], o[:], cvec_b[:], Alu.add)
                outs.append(o)
            return outs


        # memory-bank mean -> memvT [D,1] as 4 chunks
        with tc.tile_pool(name="tailmem", bufs=1) as mp:
            memx = mp.tile([128, 4 * D], f32, name="memx")
            for kc in range(4):
                dma(memx[:, kc * D:(kc + 1) * D],
                    memory_bank[kc * 128:(kc + 1) * 128, :])
            mem_ps = pool_ps.tile([1, D], f32, name="memps", tag="Tps",
                                  padded_shape=[128, 512])
            for kc in range(4):
                nc.tensor.matmul(mem_ps[:1, :], ones_sb[:],
                                 memx[:, kc * D:(kc + 1) * D],
                                 start=(kc == 0), stop=(kc == 3))
            memv = mp.tile([1, D], f32, name="memv")
            nc.vector.tensor_scalar(memv[:], mem_ps[:1, :], 1.0 / 512.0, None,
                                    Alu.mult)
            memvT = tailP.tile([128, 4], f32, name="memvT")
            for kc in range(4):
                transpose_to(memvT[:, kc:kc + 1], memv[:, kc * 128:(kc + 1) * 128],
                             f"memvT{kc}")

        with tc.tile_pool(name="tailA", bufs=1) as ta_:
            coT = transposed_cols(ta_, co, D, "coT")
            mh = big_matmul(ta_, coT, mem_w1, D, D, "memh", bias_dram=mem_b1,
                            const_lhsT=memvT)
            for g in range(NG):
                silu_(ta_, mh[g][:], mh[g][:], f"mh{g}")
            mhT = transposed_cols(ta_, mh, D, "mhT")
            mo = big_matmul(ta_, mhT, mem_w2, D, D, "memo", bias_dram=mem_b2)
            for g in range(NG):
                nc.vector.tensor_tensor(co[g][:], co[g][:], mo[g][:], Alu.add)

        # FFN: N-sharded across cores (this core's gate|val strips of 256 each)
        HWS = D // 2  # 256-wide gate and val strips
        ffn_stage = [pool_dram.tile([128, D], f32, name=f"ffn_stage{g}")
                     for g in range(NG)]
        ffn_out = [pool_dram.tile([128, D], f32, name=f"ffn_out{g}",
                                  addr_space="Shared") for g in range(NG)]
        gv = [tailP.tile([128, HWS], f32, name=f"gv{g}") for g in range(NG)]
        with tc.tile_pool(name="tailB", bufs=1) as tb_:
            coT2 = transposed_cols(tb_, co, D, "coT2")
            ff = big_matmul(tb_, coT2, up_ws, D, 2 * HWS, "ff", bias_dram=up_bs)
            for g in range(NG):
                silu_(tb_, gv[g][:], ff[g][:, :HWS], f"gv{g}")
                nc.vector.tensor_tensor(gv[g][:], gv[g][:], ff[g][:, HWS:],
                                        Alu.mult)
        with tc.tile_pool(name="tailC", bufs=1) as tcp:
            gvT = tcp.tile([128, 2 * S], f32, name="gvT")
            for g in range(NG):
                for kc in range(2):
                    transpose_to(gvT[:, kc * S + g * 128: kc * S + (g + 1) * 128],
                                 gv[g][:, kc * 128:(kc + 1) * 128], f"gvT{g}_{kc}")
            dwsb = tcp.tile([128, 2 * D], f32, name="dwsb")
            for kc in range(2):
                dma(dwsb[:, kc * D:(kc + 1) * D],
                    down_ws[kc * 128:(kc + 1) * 128, :])
            for g in range(NG):
                ps = pool_mm.tile([128, D], f32, name="Fps", tag="Fps")
                for kc in range(2):
                    nc.tensor.matmul(ps[:], gvT[:, kc * S + g * 128:
                                                 kc * S + (g + 1) * 128],
                                     dwsb[:, kc * D:(kc + 1) * D],
                                     start=(kc == 0), stop=(kc == 1))
                part = tcp.tile([128, D], f32, name=f"part{g}", tag="part")
                nc.vector.tensor_copy(part[:], ps[:])
                dma(ffn_stage[g][:], part[:])
                nc.gpsimd.collective_compute(
                    "AllReduce", Alu.add, replica_groups=RG,
                    ins=[ffn_stage[g][:]], outs=[ffn_out[g][:]])
            dnb_b = bcast_row(tcp, down_b, D, "dnb_b")
            for g in range(NG):
                fsum = tcp.tile([128, D], f32, name=f"fsum{g}", tag="fsum")
                dma(fsum[:], ffn_out[g][:])
                nc.vector.tensor_tensor(fsum[:], fsum[:], dnb_b[:], Alu.add)
                nc.vector.tensor_tensor(fsum[:], fsum[:], co[g][:], Alu.add)
                dma(out_dram[g * 128:(g + 1) * 128, :], fsum[:])

    return nc


def _install_ntff_shim():
    """Reconstitute the missing antenv.axon_hooks module so
    run_bass_kernel_spmd(trace=True) can reach the axon NTFF profiler."""
    import sys
    import types

    if "antenv.axon_hooks" in sys.modules:
        return
    import antenv

    mod = types.ModuleType("antenv.axon_hooks")
    _h = [None]
    mod.set_axon_ntff_profile_hook = lambda h: _h.__setitem__(0, h)
    mod.get_axon_ntff_profile_hook = lambda: _h[0]
    sys.modules["antenv.axon_hooks"] = mod
    antenv.axon_hooks = mod
    try:
        from trn_agent_boot.trn_boot import _ntff_profile_via_ctypes

        mod.set_axon_ntff_profile_hook(
            _ntff_profile_via_ctypes("/opt/axon/libaxon_pjrt.so"))
    except Exception:
        pass


def kernel(**inputs):
    from concourse.bass_utils import run_bass_kernel_spmd
    _install_ntff_shim()

    sin, cos, qpoly = _host_constants()
    x = np.ascontiguousarray(np.asarray(inputs["x"], np.float32).reshape(S, D))
    patterns = np.ascontiguousarray(np.asarray(inputs["flow_patterns"], np.float32))

    nc = build_kernel()
    nc.finalize()

    def a(k):
        return np.ascontiguousarray(np.asarray(inputs[k], np.float32))

    def row(k):
        return np.ascontiguousarray(np.asarray(inputs[k], np.float32).reshape(1, -1))

    up_w = np.asarray(inputs["up_w"], np.float32)      # [D, 8D]
    up_b = np.asarray(inputs["up_b"], np.float32).reshape(-1)
    down_w = np.asarray(inputs["down_w"], np.float32)  # [4D, D]

    base = {
        "x": x,
        "sel_w1": a("sel_w1"), "sel_b1": row("sel_b1"),
        "sel_w2": a("sel_w2"), "sel_b2": row("sel_b2"),
        "win_w1": a("win_w1"), "win_b1": row("win_b1"),
        "win_w2": a("win_w2"), "win_b2": row("win_b2"),
        "int_w1": a("int_w1"), "int_b1": row("int_b1"),
        "int_w2": a("int_w2"), "int_b2": row("int_b2"),
        "mem_w1": a("mem_w1"), "mem_b1": row("mem_b1"),
        "mem_w2": a("mem_w2"), "mem_b2": row("mem_b2"),
        "memory_bank": a("memory_bank"),
        "down_b": row("down_b"),
        "n1_g": row("n1_g"), "n1_b": row("n1_b"),
        "n2_g": row("n2_g"), "n2_b": row("n2_b"),
        "rope_sin": sin, "rope_cos": cos,
        "qpoly": qpoly.reshape(1, 4),
    }
    HWS = D // 2
    in_maps = []
    for c in range(NCORES):
        m = dict(base)
        m["pat_sl"] = np.ascontiguousarray(
            patterns[:, c * ISLICE:(c + 1) * ISLICE, :].reshape(P, FREE))
        gs = slice(c * HWS, (c + 1) * HWS)
        vs = slice(4 * D + c * HWS, 4 * D + (c + 1) * HWS)
        m["up_ws"] = np.ascontiguousarray(
            np.concatenate([up_w[:, gs], up_w[:, vs]], axis=1))
        m["up_bs"] = np.ascontiguousarray(
            np.concatenate([up_b[gs], up_b[vs]]).reshape(1, -1))
        m["down_ws"] = np.ascontiguousarray(down_w[c * HWS:(c + 1) * HWS, :])
        in_maps.append(m)

    trace = os.environ.get("KERNEL_TRACE", "0") == "1"
    res = run_bass_kernel_spmd(nc, in_maps, list(range(NCORES)), trace=trace)
    out0 = res.results[0]
    kernel.last_results = res.results
    kernel.last_exec_ns = getattr(res, "exec_time_ns", None)
    return out0["out"].reshape(B, S, D).astype(np.float32)


if __name__ == "__main__":
    data = np.load("/tmp/inputs.npz")
    inputs = {k: data[k] for k in data.files}
    out = kernel(**inputs)
    print("out", out.shape, float(np.abs(out).max()))
